# revision 13
# baseline (speedup 1.0000x reference)
"""BayesianLinear (y = x @ (mu + softplus(rho) * eps).T + bias) on 8 TRN2 cores.

Column-parallel sharding: each core owns OUT_F/8 = 512 output features.

Host-side prep is pure layout/precision staging (no reference math):
  - x is cast to bf16 and pre-tiled into the SBUF layout the TensorEngine
    needs for its stationary operand, in delivery-order groupings:
    k-pair chunks across tiles 0-7 (first half) / tiles 0-3 (second
    half), per-tile halves for tiles 4-7 and the extras 8-11, and
    tile-major 1MB blocks for the stream tiles 12-63.
  - weight mu/rho/eps shards are transposed to [in_f, o_sh] and packed
    per single K-block into one bf16-typed tensor (mu bf16 | eps bf16 |
    rho fp16-bits). rho ships fp16 because softplus amplifies its
    quantization ~3x.

Device per core:
  1. ONE delivery queue (sync) carries everything needed before the
     stream phase, in exact consumption order: bias, then per k-pair
     [pk_k, pk_k+1, x-chunk(k,k+1)], extras' halves ahead of their
     passes, tiles 4-7 second halves last, then the stream tiles
     (4-deep prefetch ring). A single queue avoids SDMA round-robin
     unfairness (measured: big chunks on a second queue starve the
     393KB pk singles to ~60GB/s). y rides the scalar queue where its
     eviction-wait cannot head-of-line block x prefetch.
  2. W^T construction per K-block: Exp then Ln(1+x) on ACT (one shared
     table set), mul(eps)/add(mu) on DVE at 2x bf16 rate into the
     resident W^T tile [128, 32, 512]. The DVE program interleaves
     construction with partial evictions in consumption order so PSUM
     banks free on time.
  3. PE program: K=128 dummy warmups (real array activity, so the HAM
     clock-gate opens during warmup, not mid-pass), bias broadcast,
     then split-K over 12 tiles with 8 PSUM banks:
       A:  tiles 0-7, k 0-15, k-interleaved (delivery-paced, ~1 tile
           of PE work per delivered chunk+pk pair, stalls << 3.4us so
           the HAM stays warm), bias-added partials parked in SBUF bf16
       A2: extras 8-11, k 0-15, full speed off resident W^T
       B:  extras 8-11 + tiles 0-3, k 16-31, k-interleaved (the
           delivery-paced pass rides the pk16-31 stream)
       B2: tiles 4-7, k 16-31, full speed off their late halves
     Remaining 52 tiles stream one PSUM bank each; DVE fuses eviction
     with the partial/bias add.
"""

import numpy as np
import ml_dtypes

import concourse.bacc as bacc
import concourse.mybir as mybir
import concourse.tile as tile
from concourse.bass_utils import run_bass_kernel_spmd

BATCH = 8192
IN_F = 4096
OUT_F = 4096
N_CORES = 8
P = 128

GROUP = 8
EXTRA = 4
NGRP = GROUP + EXTRA

_NC_CACHE = {}


def build_nc(batch=BATCH, in_f=IN_F, o_sh=OUT_F // N_CORES):
    KB = in_f // P  # K-blocks of 128 along the contraction dim
    BT = batch // P  # 128-row output tiles
    HALF = KB // 2
    NPAIR = HALF // 2  # k-pairs per half

    nc = bacc.Bacc(
        "TRN2",
        target_bir_lowering=False,
        debug=False,
        enable_asserts=False,
        num_devices=N_CORES,
    )
    bf16 = mybir.dt.bfloat16
    f16 = mybir.dt.float16
    f32 = mybir.dt.float32

    # x DRAM tensors, one per delivery grouping
    xga = nc.declare_dram_parameter("x_ga", [P, HALF, GROUP, P], bf16, isOutput=False)
    xgb = nc.declare_dram_parameter("x_gb", [P, HALF, 4, P], bf16, isOutput=False)
    xgc = nc.declare_dram_parameter("x_gc", [4, P, HALF, P], bf16, isOutput=False)
    xe0 = nc.declare_dram_parameter("x_e0", [EXTRA, P, HALF, P], bf16, isOutput=False)
    xe1 = nc.declare_dram_parameter("x_e1", [P, HALF, EXTRA, P], bf16, isOutput=False)
    xs = nc.declare_dram_parameter("x_s", [BT - NGRP, P, KB, P], bf16, isOutput=False)
    wpk = nc.declare_dram_parameter("wpk_t", [KB, P, 3 * o_sh], bf16, isOutput=False)
    bpk = nc.declare_dram_parameter("bias_pk", [1, 3 * o_sh], f32, isOutput=False)
    y = nc.declare_dram_parameter("y", [batch, o_sh], f32, isOutput=True)

    act_exp = mybir.ActivationFunctionType.Exp
    act_ln = mybir.ActivationFunctionType.Ln

    N_WARM = 13
    RING = 3  # pk/sp construction ring depth

    with tile.TileContext(nc) as tc:
        with (
            tc.tile_pool(name="const", bufs=1) as const,
            tc.tile_pool(name="wcons", bufs=RING) as wcons,
            tc.tile_pool(name="xin", bufs=4) as xin,
            tc.tile_pool(name="part", bufs=NGRP) as part,
            tc.tile_pool(name="yout", bufs=2) as yout,
            tc.tile_pool(name="psum", bufs=8, space="PSUM") as psum_pool,
        ):
            bias_sb = const.tile([P, o_sh], f32, tag="bias_sb")
            bias_bf = const.tile([1, o_sh], bf16, tag="bias_bf")
            ones = const.tile([1, P], bf16, tag="ones")
            nc.vector.memset(ones[:], 1.0)
            # K=128 warmup operands: full-array activity so PE_HAM's
            # SHORT window sees real busy-ness and opens the clock gate
            # during warmup (K=1 warmups don't register).
            wl = const.tile([P, P], bf16, tag="wl")
            nc.vector.memset(wl[:], 0.0)
            wr = const.tile([P, o_sh], bf16, tag="wr")
            nc.vector.memset(wr[:], 0.0)

            # Bias inputs ride the sync queue ahead of everything (6 KiB,
            # one packed partition-0 [1, 3*o_sh] DMA: mu | rho | eps).
            b_all = const.tile([1, 3 * o_sh], f32, tag="b_all")
            nc.sync.dma_start(out=b_all[:], in_=bpk[:])
            b_mu = b_all[:, 0:o_sh]
            b_rho = b_all[:, o_sh : 2 * o_sh]
            b_eps = b_all[:, 2 * o_sh : 3 * o_sh]
            b_sp = const.tile([1, o_sh], f32, tag="b_sp")
            nc.scalar.activation(b_sp[:], b_rho, act_exp)
            nc.scalar.activation(b_sp[:], b_sp[:], act_ln, bias=1.0)
            nc.vector.tensor_mul(out=b_sp[:], in0=b_sp[:], in1=b_eps)
            nc.vector.tensor_add(out=bias_bf[:], in0=b_sp[:], in1=b_mu)

            # PE warmup + bias broadcast emitted early so the DVE-side
            # bias_sb eviction precedes construction in the DVE program.
            warm_ps = psum_pool.tile([P, o_sh], f32, tag="ps", name="warm_ps")
            for w in range(N_WARM):
                nc.tensor.matmul(warm_ps[:], lhsT=wl[:], rhs=wr[:])
            bias_ps = psum_pool.tile([P, o_sh], f32, tag="ps", name="bias_ps")
            nc.tensor.matmul(bias_ps[:], lhsT=ones[:], rhs=bias_bf[:])
            nc.vector.tensor_copy(out=bias_sb[:], in_=bias_ps[:])

            WT_bf = const.tile([P, KB, o_sh], bf16, tag="WT_bf")

            # SBUF x tiles for the group phase
            ca = [
                const.tile([P, 2, GROUP, P], bf16, tag=f"ca{j}", name=f"ca{j}")
                for j in range(NPAIR)
            ]
            cbt = [
                const.tile([P, 2, 4, P], bf16, tag=f"cb{j}", name=f"cb{j}")
                for j in range(NPAIR)
            ]
            xgc_sb = [
                const.tile([P, HALF, P], bf16, tag=f"xgc{i}", name=f"xgc{i}")
                for i in range(4)
            ]
            xe0_sb = [
                const.tile([P, HALF, P], bf16, tag=f"xe0_{e}", name=f"xe0_{e}")
                for e in range(EXTRA)
            ]
            xe1_sb = const.tile([P, HALF, EXTRA, P], bf16, tag="xe1", name="xe1")

            # ---- construction helpers
            pks = []
            sps = []

            def emit_pk(k):
                pk = wcons.tile([P, 3 * o_sh], bf16, tag="pk", name=f"pk{k}")
                nc.sync.dma_start(out=pk[:], in_=wpk[k])
                pks.append(pk)
                # softplus(rho) = ln(1 + exp(rho)); Exp+Ln share one ACT
                # table set (natural_log_exp_and_others).
                rho_t = pk[:, 2 * o_sh : 3 * o_sh].bitcast(f16)
                sp_f = wcons.tile([P, o_sh], f16, tag="spf", name=f"spf{k}")
                sp_t = wcons.tile([P, o_sh], bf16, tag="sp", name=f"sp{k}")
                nc.scalar.activation(sp_f[:], rho_t[:], act_exp)
                nc.scalar.activation(sp_t[:], sp_f[:], act_ln, bias=1.0)
                sps.append(sp_t)

            def emit_wt(k):
                pk = pks[k]
                mu_t = pk[:, 0:o_sh]
                eps_t = pk[:, o_sh : 2 * o_sh]
                sp_t = sps[k]
                nc.vector.tensor_mul(out=sp_t[:], in0=sp_t[:], in1=eps_t[:])
                nc.vector.tensor_add(out=WT_bf[:, k, :], in0=sp_t[:], in1=mu_t[:])

            # ---- delivery program (sync queue), consumption order.
            # First half: [pk2j, pk2j+1, ca_j] per k-pair; extras' first
            # halves land just before pass A ends.
            for j in range(NPAIR):
                emit_pk(2 * j)
                emit_pk(2 * j + 1)
                nc.sync.dma_start(out=ca[j][:], in_=xga[:, 2 * j : 2 * j + 2])
            for e in range(EXTRA):
                nc.sync.dma_start(out=xe0_sb[e][:], in_=xe0[e])
            # extras' second halves (whole block: pass B's k16 row reads
            # all four extras at once)
            nc.sync.dma_start(out=xe1_sb[:], in_=xe1[:])
            # Second half: [pk, pk, cb_j] pairs for tiles 0-3, then
            # tiles 4-7 per-tile halves (consumed last, full speed).
            for j in range(NPAIR):
                emit_pk(HALF + 2 * j)
                emit_pk(HALF + 2 * j + 1)
                nc.sync.dma_start(out=cbt[j][:], in_=xgb[:, 2 * j : 2 * j + 2])
            for i in range(4):
                nc.sync.dma_start(out=xgc_sb[i][:], in_=xgc[i])

            def glhs(i, k):
                """lhsT AP for group tile i (0..NGRP-1), k-block k."""
                if k < HALF:
                    if i < GROUP:
                        return ca[k // 2][:, k % 2, i, :]
                    return xe0_sb[i - GROUP][:, k, :]
                if i < 4:
                    return cbt[(k - HALF) // 2][:, (k - HALF) % 2, i, :]
                if i < GROUP:
                    return xgc_sb[i - 4][:, k - HALF, :]
                return xe1_sb[:, k - HALF, i - GROUP, :]

            # ---- DVE construction for the first half (pass-A weights)
            for k in range(HALF):
                emit_wt(k)

            # Pass A: tiles 0-7, k 0..15, k-interleaved.
            pss = [
                psum_pool.tile([P, o_sh], f32, tag="ps", name=f"ps_a{bt}")
                for bt in range(GROUP)
            ]
            for k in range(HALF):
                for i in range(GROUP):
                    nc.tensor.matmul(
                        pss[i][:],
                        lhsT=glhs(i, k),
                        rhs=WT_bf[:, k, :],
                        start=(k == 0),
                        stop=(k == HALF - 1),
                    )
            parts = {}
            for i in range(GROUP):
                pa = part.tile([P, o_sh], bf16, tag="pA", name=f"pA_{i}")
                nc.vector.tensor_add(out=pa[:], in0=pss[i][:], in1=bias_sb[:])
                parts[i] = pa
            # A2: extras' first halves at full speed (W^T 0..HALF
            # resident); second-half constructions interleave on DVE so
            # each partsA2 eviction stays unblocked.
            wt_next = HALF
            for e in range(GROUP, NGRP):
                emit_wt(wt_next)
                emit_wt(wt_next + 1)
                wt_next += 2
                ps = psum_pool.tile([P, o_sh], f32, tag="ps", name=f"ps_a{e}")
                for k in range(HALF):
                    nc.tensor.matmul(
                        ps[:],
                        lhsT=glhs(e, k),
                        rhs=WT_bf[:, k, :],
                        start=(k == 0),
                        stop=(k == HALF - 1),
                    )
                pa = part.tile([P, o_sh], bf16, tag="pA", name=f"pA_{e}")
                nc.vector.tensor_add(out=pa[:], in0=ps[:], in1=bias_sb[:])
                parts[e] = pa
            for k in range(wt_next, KB):
                emit_wt(k)

            def split_tail(ps, pa, bt):
                y_sb = yout.tile([P, o_sh], f32, tag="y_sb")
                nc.vector.tensor_add(out=y_sb[:], in0=ps[:], in1=pa[:])
                nc.scalar.dma_start(out=y[bt * P : (bt + 1) * P, :], in_=y_sb[:])

            # B: extras 8-11 + tiles 0-3, k 16..31, k-interleaved — the
            # delivery-paced pass rides the pk16-31 + cb chunk stream.
            b_tiles = [8, 9, 10, 11, 0, 1, 2, 3]
            psb = {
                i: psum_pool.tile([P, o_sh], f32, tag="ps", name=f"ps_b{i}")
                for i in b_tiles
            }
            for k in range(HALF, KB):
                for i in b_tiles:
                    nc.tensor.matmul(
                        psb[i][:],
                        lhsT=glhs(i, k),
                        rhs=WT_bf[:, k, :],
                        start=(k == HALF),
                        stop=(k == KB - 1),
                    )
            for i in b_tiles:
                split_tail(psb[i], parts[i], i)
            # B2: tiles 4-7 second halves at full speed
            for i in range(4, GROUP):
                ps = psum_pool.tile([P, o_sh], f32, tag="ps", name=f"ps_b{i}")
                for k in range(HALF, KB):
                    nc.tensor.matmul(
                        ps[:],
                        lhsT=glhs(i, k),
                        rhs=WT_bf[:, k, :],
                        start=(k == HALF),
                        stop=(k == KB - 1),
                    )
                split_tail(ps, parts[i], i)

            # ---- remaining tiles stream one PSUM bank each off a
            # 4-deep prefetch ring on the sync queue.
            for bt in range(NGRP, BT):
                xbf_t = xin.tile([P, KB, P], bf16, tag="xT")
                nc.sync.dma_start(out=xbf_t[:], in_=xs[bt - NGRP])
                ps = psum_pool.tile([P, o_sh], f32, tag="ps")
                for k in range(KB):
                    nc.tensor.matmul(
                        ps[:],
                        lhsT=xbf_t[:, k, :],
                        rhs=WT_bf[:, k, :],
                        start=(k == 0),
                        stop=(k == KB - 1),
                    )
                y_sb = yout.tile([P, o_sh], f32, tag="y_sb")
                nc.vector.tensor_add(out=y_sb[:], in0=ps[:], in1=bias_sb[:])
                nc.scalar.dma_start(out=y[bt * P : (bt + 1) * P, :], in_=y_sb[:])

    # Skip bacc's pre-placed InstLoadActFuncSet: on large graphs walrus's
    # parallel-pass fork can separate the hoisted load from its activations
    # ("No Act func set exist for this instruction"); walrus's own lower_act
    # placement handles forked subgraphs correctly.
    nc.insert_act_table_loads = lambda: None
    nc.compile()
    return nc


def _prep_x(x):
    """[batch, in_f] fp32 -> bf16 tiles with layouts matching the DRAM
    tensors (x_t[..., pi, ..., bi] = x[bt*128 + bi, po*128 + pi]):
      xga [P, HALF, GROUP, P]  tiles 0-7,  k 0..15, k-major
      xgb [P, HALF, 4, P]      tiles 0-3,  k 16..31, k-major
      xgc [4, P, HALF, P]      tiles 4-7,  k 16..31, tile-major
      xe0 [EXTRA, P, HALF, P]  extras 8-11, k 0..15, tile-major
      xe1 [P, HALF, EXTRA, P]  extras 8-11, k 16..31, k-major
      xs  [BT-NGRP, P, KB, P]  stream tiles, tile-major
    """
    batch, in_f = x.shape
    KB = in_f // P
    HALF = KB // 2
    BT = batch // P
    xbf = x.astype(ml_dtypes.bfloat16)
    xbf = xbf.reshape(BT, P, KB, P)  # [bt, bi, po, pi]
    xt = xbf.transpose(0, 3, 2, 1)  # [bt, pi, po, bi]
    xga = np.ascontiguousarray(xt[:GROUP, :, :HALF].transpose(1, 2, 0, 3))
    xgb = np.ascontiguousarray(xt[:4, :, HALF:].transpose(1, 2, 0, 3))
    xgc = np.ascontiguousarray(xt[4:GROUP, :, HALF:])
    xe0 = np.ascontiguousarray(xt[GROUP:NGRP, :, :HALF])
    xe1 = np.ascontiguousarray(xt[GROUP:NGRP, :, HALF:].transpose(1, 2, 0, 3))
    xs = np.ascontiguousarray(xt[NGRP:])
    return xga, xgb, xgc, xe0, xe1, xs


def _tile_w(w, dtype):
    """[o_sh, in_f] -> tiled [KB, 128, o_sh] with w_t[k, pi, o] = w[o, k*128 + pi]."""
    o_sh, in_f = w.shape
    return np.ascontiguousarray(w.T.reshape(in_f // P, P, o_sh)).astype(dtype)


def _prep_wpk(wmu, wrho, weps):
    """Pack mu (bf16), eps (bf16), rho (fp16 bits viewed as bf16) into one
    bf16-typed [KB, 128, 3*o_sh] tensor — one DMA per K-block."""
    mu = _tile_w(wmu, ml_dtypes.bfloat16)
    eps = _tile_w(weps, ml_dtypes.bfloat16)
    rho = _tile_w(wrho, np.float16).view(ml_dtypes.bfloat16)
    return np.ascontiguousarray(np.concatenate([mu, eps, rho], axis=2))


def make_in_maps(x, weight_mu, weight_rho, bias_mu, bias_rho, weight_eps, bias_eps):
    o_sh = OUT_F // N_CORES
    xga, xgb, xgc, xe0, xe1, xs = _prep_x(np.asarray(x, dtype=np.float32))
    wmu = np.asarray(weight_mu, dtype=np.float32)
    wrho = np.asarray(weight_rho, dtype=np.float32)
    weps = np.asarray(weight_eps, dtype=np.float32)
    bpk = np.stack(
        [
            np.asarray(bias_mu, dtype=np.float32),
            np.asarray(bias_rho, dtype=np.float32),
            np.asarray(bias_eps, dtype=np.float32),
        ]
    )  # [3, OUT_F]

    in_maps = []
    for c in range(N_CORES):
        rs = slice(c * o_sh, (c + 1) * o_sh)
        in_maps.append(
            {
                "x_ga": xga,
                "x_gb": xgb,
                "x_gc": xgc,
                "x_e0": xe0,
                "x_e1": xe1,
                "x_s": xs,
                "wpk_t": _prep_wpk(wmu[rs], wrho[rs], weps[rs]),
                "bias_pk": np.ascontiguousarray(bpk[:, rs].reshape(1, -1)),
            }
        )
    return in_maps


def kernel(x, weight_mu, weight_rho, bias_mu, bias_rho, weight_eps, bias_eps):
    o_sh = OUT_F // N_CORES
    key = (x.shape, o_sh)
    if key not in _NC_CACHE:
        _NC_CACHE[key] = build_nc(x.shape[0], x.shape[1], o_sh)
    nc = _NC_CACHE[key]

    in_maps = make_in_maps(
        x, weight_mu, weight_rho, bias_mu, bias_rho, weight_eps, bias_eps
    )
    res = run_bass_kernel_spmd(nc, in_maps, core_ids=list(range(N_CORES)))
    return np.concatenate([res.results[c]["y"] for c in range(N_CORES)], axis=1)


# revision 14
# speedup vs baseline: 1.0381x; 1.0381x over previous
"""BayesianLinear (y = x @ (mu + softplus(rho) * eps).T + bias) on 8 TRN2 cores.

Column-parallel sharding: each core owns OUT_F/8 = 512 output features.

Host-side prep is pure layout/precision staging (no reference math):
  - x is cast to bf16 and pre-tiled into the SBUF layout the TensorEngine
    needs for its stationary operand, in delivery-order groupings:
    k-pair chunks across tiles (ca: tiles 0-7 first half, cb: tiles 0-1
    second half), per-tile halves (extras 8-13 first halves, tiles 2-7
    second halves, stream tiles), k-major block for extras' second
    halves.
  - weight mu/rho/eps shards are transposed to [in_f, o_sh] and packed
    per single K-block into one bf16-typed tensor (mu bf16 | eps bf16 |
    rho fp16-bits). rho ships fp16 because softplus amplifies its
    quantization ~3x.

Device per core (phase 1 sized so PE work >= delivery time at the
~290 GB/s the HBM actually delivers: 14 parked tiles x 6.9us = 97us of
PE work vs 27.7 MB of phase-1 bytes = 96us):
  1. Delivery: sync queue carries [bias, ca0, ca1, then per k-pair
     pk,pk,chunk in consumption order, extras/B2 halves, stream tiles];
     the scalar queue carries ONLY pk0-3 (a parallel fast start while
     it has nothing else — its first ACT can't run before the ~2.7us
     table load anyway) and later the y writes, whose eviction-wait
     must not head-of-line block x prefetch. A single main queue
     avoids SDMA round-robin unfairness (measured: big chunks on a
     second queue starve 393KB pk singles to ~60GB/s).
  2. W^T construction per K-block: Exp then Ln(1+x) on ACT (one shared
     table set), mul(eps)/add(mu) on DVE at 2x bf16 rate into the
     resident W^T tile [128, 32, 512]. The DVE program interleaves
     construction with partial evictions in consumption order so PSUM
     banks free on time.
  3. PE program: K=128 dummy warmups (real array activity, so the HAM
     clock-gate opens during warmup — K=1 warmups don't register and
     the first ~4us of real matmuls would run at 1.2 GHz), bias
     broadcast, then split-K over 14 tiles with 8 PSUM banks:
       A:  tiles 0-7, k 0-15, k-interleaved, delivery-paced with
           per-k-pair micro-stalls << 3.4us (so the HAM never
           re-throttles), bias-added partials parked in SBUF bf16
       A2: extras 8-13, k 0-15, full speed off resident W^T
       B:  extras 8-13 + tiles 0-1, k 16-31, k-interleaved riding the
           pk16-31 stream
       B2: tiles 2-7, k 16-31, full speed off late-arriving halves
     Remaining 50 tiles stream one PSUM bank each; DVE fuses eviction
     with the partial/bias add.
"""

import numpy as np
import ml_dtypes

import concourse.bacc as bacc
import concourse.mybir as mybir
import concourse.tile as tile
from concourse.bass_utils import run_bass_kernel_spmd

BATCH = 8192
IN_F = 4096
OUT_F = 4096
N_CORES = 8
P = 128

GROUP = 8
EXTRA = 6
NGRP = GROUP + EXTRA
NB1 = 2  # tiles 0..NB1-1 join the interleaved pass B; tiles NB1..7 are B2

_NC_CACHE = {}


def build_nc(batch=BATCH, in_f=IN_F, o_sh=OUT_F // N_CORES):
    KB = in_f // P  # K-blocks of 128 along the contraction dim
    BT = batch // P  # 128-row output tiles
    HALF = KB // 2
    NPAIR = HALF // 2  # k-pairs per half

    nc = bacc.Bacc(
        "TRN2",
        target_bir_lowering=False,
        debug=False,
        enable_asserts=False,
        num_devices=N_CORES,
    )
    bf16 = mybir.dt.bfloat16
    f16 = mybir.dt.float16
    f32 = mybir.dt.float32

    # x DRAM tensors, one per delivery grouping
    xga = nc.declare_dram_parameter("x_ga", [P, HALF, GROUP, P], bf16, isOutput=False)
    xgb = nc.declare_dram_parameter("x_gb", [P, HALF, NB1, P], bf16, isOutput=False)
    xgc = nc.declare_dram_parameter(
        "x_gc", [GROUP - NB1, P, HALF, P], bf16, isOutput=False
    )
    xe0 = nc.declare_dram_parameter("x_e0", [EXTRA, P, HALF, P], bf16, isOutput=False)
    xe1 = nc.declare_dram_parameter("x_e1", [P, HALF, EXTRA, P], bf16, isOutput=False)
    xs = nc.declare_dram_parameter("x_s", [BT - NGRP, P, KB, P], bf16, isOutput=False)
    wpk = nc.declare_dram_parameter("wpk_t", [KB, P, 3 * o_sh], bf16, isOutput=False)
    bpk = nc.declare_dram_parameter("bias_pk", [1, 3 * o_sh], f32, isOutput=False)
    y = nc.declare_dram_parameter("y", [batch, o_sh], f32, isOutput=True)

    act_exp = mybir.ActivationFunctionType.Exp
    act_ln = mybir.ActivationFunctionType.Ln

    N_WARM = 13
    RING = 4  # pk/sp construction ring depth (pk0-3 fire unringed on scalar)

    with tile.TileContext(nc) as tc:
        with (
            tc.tile_pool(name="const", bufs=1) as const,
            tc.tile_pool(name="wcons", bufs=RING) as wcons,
            tc.tile_pool(name="xin", bufs=8) as xin,
            tc.tile_pool(name="part", bufs=NGRP) as part,
            tc.tile_pool(name="yout", bufs=2) as yout,
            tc.tile_pool(name="psum", bufs=8, space="PSUM") as psum_pool,
        ):
            bias_sb = const.tile([P, o_sh], f32, tag="bias_sb")
            bias_bf = const.tile([1, o_sh], bf16, tag="bias_bf")
            ones = const.tile([1, P], bf16, tag="ones")
            nc.vector.memset(ones[:], 1.0)
            # K=128 warmup operands: full-array activity so PE_HAM's
            # SHORT window sees real busy-ness and opens the clock gate
            # during warmup (K=1 warmups don't register).
            wl = const.tile([P, P], bf16, tag="wl")
            nc.vector.memset(wl[:], 0.0)
            wr = const.tile([P, o_sh], bf16, tag="wr")
            nc.vector.memset(wr[:], 0.0)

            # ---- construction helpers
            pks = []
            sps = []

            def emit_pk(k, eng):
                pk = wcons.tile([P, 3 * o_sh], bf16, tag="pk", name=f"pk{k}")
                eng.dma_start(out=pk[:], in_=wpk[k])
                pks.append(pk)
                # softplus(rho) = ln(1 + exp(rho)); Exp+Ln share one ACT
                # table set (natural_log_exp_and_others).
                rho_t = pk[:, 2 * o_sh : 3 * o_sh].bitcast(f16)
                sp_f = wcons.tile([P, o_sh], f16, tag="spf", name=f"spf{k}")
                sp_t = wcons.tile([P, o_sh], bf16, tag="sp", name=f"sp{k}")
                nc.scalar.activation(sp_f[:], rho_t[:], act_exp)
                nc.scalar.activation(sp_t[:], sp_f[:], act_ln, bias=1.0)
                sps.append(sp_t)

            def emit_wt(k):
                pk = pks[k]
                mu_t = pk[:, 0:o_sh]
                eps_t = pk[:, o_sh : 2 * o_sh]
                sp_t = sps[k]
                nc.vector.tensor_mul(out=sp_t[:], in0=sp_t[:], in1=eps_t[:])
                nc.vector.tensor_add(out=WT_bf[:, k, :], in0=sp_t[:], in1=mu_t[:])

            # pk0-3 lead on the (otherwise idle) scalar queue: weights
            # for the first rows arrive in parallel with bias+ca0 on
            # sync. RING=4 so none of these gate on construction.
            emit_pk(0, nc.scalar)
            emit_pk(1, nc.scalar)

            # Bias inputs ride the sync queue ahead of everything (6 KiB,
            # one packed partition-0 [1, 3*o_sh] DMA: mu | rho | eps).
            b_all = const.tile([1, 3 * o_sh], f32, tag="b_all")
            nc.sync.dma_start(out=b_all[:], in_=bpk[:])
            b_mu = b_all[:, 0:o_sh]
            b_rho = b_all[:, o_sh : 2 * o_sh]
            b_eps = b_all[:, 2 * o_sh : 3 * o_sh]
            b_sp = const.tile([1, o_sh], f32, tag="b_sp")
            nc.scalar.activation(b_sp[:], b_rho, act_exp)
            nc.scalar.activation(b_sp[:], b_sp[:], act_ln, bias=1.0)
            nc.vector.tensor_mul(out=b_sp[:], in0=b_sp[:], in1=b_eps)
            nc.vector.tensor_add(out=bias_bf[:], in0=b_sp[:], in1=b_mu)

            # PE warmup + bias broadcast emitted early so the DVE-side
            # bias_sb eviction precedes construction in the DVE program.
            warm_ps = psum_pool.tile([P, o_sh], f32, tag="ps", name="warm_ps")
            for w in range(N_WARM):
                nc.tensor.matmul(warm_ps[:], lhsT=wl[:], rhs=wr[:])
            bias_ps = psum_pool.tile([P, o_sh], f32, tag="ps", name="bias_ps")
            nc.tensor.matmul(bias_ps[:], lhsT=ones[:], rhs=bias_bf[:])
            nc.vector.tensor_copy(out=bias_sb[:], in_=bias_ps[:])

            WT_bf = const.tile([P, KB, o_sh], bf16, tag="WT_bf")

            # SBUF x tiles for the group phase
            ca = [
                const.tile([P, 2, GROUP, P], bf16, tag=f"ca{j}", name=f"ca{j}")
                for j in range(NPAIR)
            ]
            cbt = [
                const.tile([P, 2, NB1, P], bf16, tag=f"cb{j}", name=f"cb{j}")
                for j in range(NPAIR)
            ]
            xe0_sb = [
                const.tile([P, HALF, P], bf16, tag=f"xe0_{e}", name=f"xe0_{e}")
                for e in range(EXTRA)
            ]
            xe1_sb = const.tile([P, HALF, EXTRA, P], bf16, tag="xe1", name="xe1")

            # ---- delivery program (sync queue), consumption order.
            nc.sync.dma_start(out=ca[0][:], in_=xga[:, 0:2])
            nc.sync.dma_start(out=ca[1][:], in_=xga[:, 2:4])
            emit_pk(2, nc.scalar)
            emit_pk(3, nc.scalar)
            for j in range(2, NPAIR):
                emit_pk(2 * j, nc.sync)
                emit_pk(2 * j + 1, nc.sync)
                nc.sync.dma_start(out=ca[j][:], in_=xga[:, 2 * j : 2 * j + 2])
            for e in range(EXTRA):
                nc.sync.dma_start(out=xe0_sb[e][:], in_=xe0[e])
            # extras' second halves (whole block: pass B's k16 row reads
            # all extras at once)
            nc.sync.dma_start(out=xe1_sb[:], in_=xe1[:])
            # Second half: [pk, pk, cb_j] pairs for tiles 0..NB1-1.
            for j in range(NPAIR):
                emit_pk(HALF + 2 * j, nc.sync)
                emit_pk(HALF + 2 * j + 1, nc.sync)
                nc.sync.dma_start(out=cbt[j][:], in_=xgb[:, 2 * j : 2 * j + 2])

            def glhs(i, k):
                """lhsT AP for group tile i (0..NGRP-1), k-block k
                (B2 tiles 2..7 second halves are passed explicitly)."""
                if k < HALF:
                    if i < GROUP:
                        return ca[k // 2][:, k % 2, i, :]
                    return xe0_sb[i - GROUP][:, k, :]
                if i < NB1:
                    return cbt[(k - HALF) // 2][:, (k - HALF) % 2, i, :]
                assert i >= GROUP
                return xe1_sb[:, k - HALF, i - GROUP, :]

            # ---- DVE construction for the first half (pass-A weights)
            for k in range(HALF):
                emit_wt(k)

            # Pass A: tiles 0-7, k 0..15, k-interleaved.
            pss = [
                psum_pool.tile([P, o_sh], f32, tag="ps", name=f"ps_a{bt}")
                for bt in range(GROUP)
            ]
            for k in range(HALF):
                for i in range(GROUP):
                    nc.tensor.matmul(
                        pss[i][:],
                        lhsT=glhs(i, k),
                        rhs=WT_bf[:, k, :],
                        start=(k == 0),
                        stop=(k == HALF - 1),
                    )
            parts = {}
            for i in range(GROUP):
                pa = part.tile([P, o_sh], bf16, tag="pA", name=f"pA_{i}")
                nc.vector.tensor_add(out=pa[:], in0=pss[i][:], in1=bias_sb[:])
                parts[i] = pa
            # A2: extras' first halves at full speed (W^T 0..HALF
            # resident); second-half constructions interleave on DVE so
            # each partsA2 eviction stays unblocked.
            wt_next = HALF
            for e in range(GROUP, NGRP):
                if wt_next < KB:
                    emit_wt(wt_next)
                    emit_wt(wt_next + 1)
                    wt_next += 2
                ps = psum_pool.tile([P, o_sh], f32, tag="ps", name=f"ps_a{e}")
                for k in range(HALF):
                    nc.tensor.matmul(
                        ps[:],
                        lhsT=glhs(e, k),
                        rhs=WT_bf[:, k, :],
                        start=(k == 0),
                        stop=(k == HALF - 1),
                    )
                pa = part.tile([P, o_sh], bf16, tag="pA", name=f"pA_{e}")
                nc.vector.tensor_add(out=pa[:], in0=ps[:], in1=bias_sb[:])
                parts[e] = pa
            for k in range(wt_next, KB):
                emit_wt(k)

            def split_tail(ps, pa, bt):
                y_sb = yout.tile([P, o_sh], f32, tag="y_sb")
                nc.vector.tensor_add(out=y_sb[:], in0=ps[:], in1=pa[:])
                nc.scalar.dma_start(out=y[bt * P : (bt + 1) * P, :], in_=y_sb[:])

            # B: extras + tiles 0..NB1-1, k 16..31, k-interleaved — the
            # delivery-paced pass rides the pk16-31 + cb chunk stream.
            b_tiles = list(range(GROUP, NGRP)) + list(range(NB1))
            psb = {
                i: psum_pool.tile([P, o_sh], f32, tag="ps", name=f"ps_b{i}")
                for i in b_tiles
            }
            for k in range(HALF, KB):
                for i in b_tiles:
                    nc.tensor.matmul(
                        psb[i][:],
                        lhsT=glhs(i, k),
                        rhs=WT_bf[:, k, :],
                        start=(k == HALF),
                        stop=(k == KB - 1),
                    )
            for i in b_tiles:
                split_tail(psb[i], parts[i], i)
            # B2: tiles NB1..7 second halves at full speed, off halves
            # delivered through the xin ring (its first allocations).
            for i in range(NB1, GROUP):
                hb = xin.tile([P, HALF, P], bf16, tag="xh", name=f"hb{i}")
                nc.sync.dma_start(out=hb[:], in_=xgc[i - NB1])
                ps = psum_pool.tile([P, o_sh], f32, tag="ps", name=f"ps_b{i}")
                for k in range(HALF, KB):
                    nc.tensor.matmul(
                        ps[:],
                        lhsT=hb[:, k - HALF, :],
                        rhs=WT_bf[:, k, :],
                        start=(k == HALF),
                        stop=(k == KB - 1),
                    )
                split_tail(ps, parts[i], i)

            # ---- remaining tiles stream one PSUM bank each; x arrives
            # as two half-tile DMAs through the xin ring.
            for bt in range(NGRP, BT):
                ha = xin.tile([P, HALF, P], bf16, tag="xh")
                hb = xin.tile([P, HALF, P], bf16, tag="xh")
                nc.sync.dma_start(out=ha[:], in_=xs[bt - NGRP, :, :HALF])
                nc.sync.dma_start(out=hb[:], in_=xs[bt - NGRP, :, HALF:])
                ps = psum_pool.tile([P, o_sh], f32, tag="ps")
                for k in range(KB):
                    xh = ha if k < HALF else hb
                    nc.tensor.matmul(
                        ps[:],
                        lhsT=xh[:, k % HALF, :],
                        rhs=WT_bf[:, k, :],
                        start=(k == 0),
                        stop=(k == KB - 1),
                    )
                y_sb = yout.tile([P, o_sh], f32, tag="y_sb")
                nc.vector.tensor_add(out=y_sb[:], in0=ps[:], in1=bias_sb[:])
                nc.scalar.dma_start(out=y[bt * P : (bt + 1) * P, :], in_=y_sb[:])

    # Skip bacc's pre-placed InstLoadActFuncSet: on large graphs walrus's
    # parallel-pass fork can separate the hoisted load from its activations
    # ("No Act func set exist for this instruction"); walrus's own lower_act
    # placement handles forked subgraphs correctly.
    nc.insert_act_table_loads = lambda: None
    nc.compile()
    return nc


def _prep_x(x):
    """[batch, in_f] fp32 -> bf16 tiles with layouts matching the DRAM
    tensors (x_t[..., pi, ..., bi] = x[bt*128 + bi, po*128 + pi]):
      xga [P, HALF, GROUP, P]    tiles 0-7,  k 0..15, k-major
      xgb [P, HALF, NB1, P]      tiles 0-1,  k 16..31, k-major
      xgc [6, P, HALF, P]        tiles 2-7,  k 16..31, tile-major
      xe0 [EXTRA, P, HALF, P]    extras 8-13, k 0..15, tile-major
      xe1 [P, HALF, EXTRA, P]    extras 8-13, k 16..31, k-major
      xs  [BT-NGRP, P, KB, P]    stream tiles, tile-major
    """
    batch, in_f = x.shape
    KB = in_f // P
    HALF = KB // 2
    BT = batch // P
    xbf = x.astype(ml_dtypes.bfloat16)
    xbf = xbf.reshape(BT, P, KB, P)  # [bt, bi, po, pi]
    xt = xbf.transpose(0, 3, 2, 1)  # [bt, pi, po, bi]
    xga = np.ascontiguousarray(xt[:GROUP, :, :HALF].transpose(1, 2, 0, 3))
    xgb = np.ascontiguousarray(xt[:NB1, :, HALF:].transpose(1, 2, 0, 3))
    xgc = np.ascontiguousarray(xt[NB1:GROUP, :, HALF:])
    xe0 = np.ascontiguousarray(xt[GROUP:NGRP, :, :HALF])
    xe1 = np.ascontiguousarray(xt[GROUP:NGRP, :, HALF:].transpose(1, 2, 0, 3))
    xs = np.ascontiguousarray(xt[NGRP:])
    return xga, xgb, xgc, xe0, xe1, xs


def _tile_w(w, dtype):
    """[o_sh, in_f] -> tiled [KB, 128, o_sh] with w_t[k, pi, o] = w[o, k*128 + pi]."""
    o_sh, in_f = w.shape
    return np.ascontiguousarray(w.T.reshape(in_f // P, P, o_sh)).astype(dtype)


def _prep_wpk(wmu, wrho, weps):
    """Pack mu (bf16), eps (bf16), rho (fp16 bits viewed as bf16) into one
    bf16-typed [KB, 128, 3*o_sh] tensor — one DMA per K-block."""
    mu = _tile_w(wmu, ml_dtypes.bfloat16)
    eps = _tile_w(weps, ml_dtypes.bfloat16)
    rho = _tile_w(wrho, np.float16).view(ml_dtypes.bfloat16)
    return np.ascontiguousarray(np.concatenate([mu, eps, rho], axis=2))


def make_in_maps(x, weight_mu, weight_rho, bias_mu, bias_rho, weight_eps, bias_eps):
    o_sh = OUT_F // N_CORES
    xga, xgb, xgc, xe0, xe1, xs = _prep_x(np.asarray(x, dtype=np.float32))
    wmu = np.asarray(weight_mu, dtype=np.float32)
    wrho = np.asarray(weight_rho, dtype=np.float32)
    weps = np.asarray(weight_eps, dtype=np.float32)
    bpk = np.stack(
        [
            np.asarray(bias_mu, dtype=np.float32),
            np.asarray(bias_rho, dtype=np.float32),
            np.asarray(bias_eps, dtype=np.float32),
        ]
    )  # [3, OUT_F]

    in_maps = []
    for c in range(N_CORES):
        rs = slice(c * o_sh, (c + 1) * o_sh)
        in_maps.append(
            {
                "x_ga": xga,
                "x_gb": xgb,
                "x_gc": xgc,
                "x_e0": xe0,
                "x_e1": xe1,
                "x_s": xs,
                "wpk_t": _prep_wpk(wmu[rs], wrho[rs], weps[rs]),
                "bias_pk": np.ascontiguousarray(bpk[:, rs].reshape(1, -1)),
            }
        )
    return in_maps


def kernel(x, weight_mu, weight_rho, bias_mu, bias_rho, weight_eps, bias_eps):
    o_sh = OUT_F // N_CORES
    key = (x.shape, o_sh)
    if key not in _NC_CACHE:
        _NC_CACHE[key] = build_nc(x.shape[0], x.shape[1], o_sh)
    nc = _NC_CACHE[key]

    in_maps = make_in_maps(
        x, weight_mu, weight_rho, bias_mu, bias_rho, weight_eps, bias_eps
    )
    res = run_bass_kernel_spmd(nc, in_maps, core_ids=list(range(N_CORES)))
    return np.concatenate([res.results[c]["y"] for c in range(N_CORES)], axis=1)


# revision 17
# speedup vs baseline: 1.0434x; 1.0051x over previous
"""BayesianLinear (y = x @ (mu + softplus(rho) * eps).T + bias) on 8 TRN2 cores.

Column-parallel sharding: each core owns OUT_F/8 = 512 output features.

Host-side prep is pure layout/precision staging (no reference math):
  - x is cast to bf16 and pre-tiled into the SBUF layout the TensorEngine
    needs for its stationary operand, in delivery-order groupings:
    k-pair chunks across tiles (ca: tiles 0-7 first half, cb: tiles 0-1
    second half), per-tile halves (extras 8-13 first halves, tiles 2-7
    second halves, stream tiles), k-major block for extras' second
    halves.
  - weight mu/rho/eps shards are transposed to [in_f, o_sh] and packed
    per single K-block into one bf16-typed tensor (mu bf16 | eps bf16 |
    rho fp16-bits). rho ships fp16 because softplus amplifies its
    quantization ~3x.

Device per core (phase 1 sized so PE work >= delivery time at the
~290 GB/s the HBM actually delivers: 14 parked tiles x 6.9us = 97us of
PE work vs 27.7 MB of phase-1 bytes = 96us):
  1. Delivery: sync queue carries [bias, ca0, ca1, then per k-pair
     pk,pk,chunk in consumption order, extras/B2 halves, stream tiles];
     the scalar queue carries ONLY pk0-3 (a parallel fast start while
     it has nothing else — its first ACT can't run before the ~2.7us
     table load anyway) and later the y writes, whose eviction-wait
     must not head-of-line block x prefetch. A single main queue
     avoids SDMA round-robin unfairness (measured: big chunks on a
     second queue starve 393KB pk singles to ~60GB/s).
  2. W^T construction per K-block: Exp then Ln(1+x) on ACT (one shared
     table set), mul(eps)/add(mu) on DVE at 2x bf16 rate into the
     resident W^T tile [128, 32, 512]. The DVE program interleaves
     construction with partial evictions in consumption order so PSUM
     banks free on time.
  3. PE program: K=128 dummy warmups (real array activity, so the HAM
     clock-gate opens during warmup — K=1 warmups don't register and
     the first ~4us of real matmuls would run at 1.2 GHz), bias
     broadcast, then split-K over 14 tiles with 8 PSUM banks:
       A:  tiles 0-7, k 0-15, k-interleaved, delivery-paced with
           per-k-pair micro-stalls << 3.4us (so the HAM never
           re-throttles), bias-added partials parked in SBUF bf16
       A2: extras 8-13, k 0-15, full speed off resident W^T
       B:  extras 8-13 + tiles 0-1, k 16-31, k-interleaved riding the
           pk16-31 stream
       B2: tiles 2-7, k 16-31, full speed off late-arriving halves
     Remaining 50 tiles stream one PSUM bank each; DVE fuses eviction
     with the partial/bias add.
"""

import numpy as np
import ml_dtypes

import concourse.bacc as bacc
import concourse.mybir as mybir
import concourse.tile as tile
from concourse.bass_utils import run_bass_kernel_spmd

BATCH = 8192
IN_F = 4096
OUT_F = 4096
N_CORES = 8
P = 128

GROUP = 8
EXTRA = 6
NGRP = GROUP + EXTRA
NB1 = 2  # tiles 0..NB1-1 join the interleaved pass B; tiles NB1..7 are B2

_NC_CACHE = {}


def build_nc(batch=BATCH, in_f=IN_F, o_sh=OUT_F // N_CORES):
    KB = in_f // P  # K-blocks of 128 along the contraction dim
    BT = batch // P  # 128-row output tiles
    HALF = KB // 2
    NPAIR = HALF // 2  # k-pairs per half

    nc = bacc.Bacc(
        "TRN2",
        target_bir_lowering=False,
        debug=False,
        enable_asserts=False,
        num_devices=N_CORES,
    )
    bf16 = mybir.dt.bfloat16
    f16 = mybir.dt.float16
    f32 = mybir.dt.float32

    # x DRAM tensors, one per delivery grouping
    xga = nc.declare_dram_parameter("x_ga", [P, HALF, GROUP, P], bf16, isOutput=False)
    xgb = nc.declare_dram_parameter("x_gb", [P, HALF, NB1, P], bf16, isOutput=False)
    xgc = nc.declare_dram_parameter(
        "x_gc", [GROUP - NB1, P, HALF, P], bf16, isOutput=False
    )
    xe0 = nc.declare_dram_parameter("x_e0", [EXTRA, P, HALF, P], bf16, isOutput=False)
    xe1 = nc.declare_dram_parameter("x_e1", [P, HALF, EXTRA, P], bf16, isOutput=False)
    xs = nc.declare_dram_parameter("x_s", [BT - NGRP, P, KB, P], bf16, isOutput=False)
    wpk = nc.declare_dram_parameter("wpk_t", [KB, P, 3 * o_sh], bf16, isOutput=False)
    bpk = nc.declare_dram_parameter("bias_pk", [1, 3 * o_sh], f32, isOutput=False)
    y = nc.declare_dram_parameter("y", [batch, o_sh], f32, isOutput=True)

    act_exp = mybir.ActivationFunctionType.Exp
    act_ln = mybir.ActivationFunctionType.Ln

    N_WARM = 22
    # pk ring deep enough that its trigger gates (wait wt_{k-RINGPK})
    # never bind at runtime — a gated trigger blocks the whole HWDGE
    # queue head behind it. sp ring shallow (construction-local).
    RINGPK = 6
    RINGSP = 3

    with tile.TileContext(nc) as tc:
        with (
            tc.tile_pool(name="const", bufs=1) as const,
            tc.tile_pool(name="wpkp", bufs=RINGPK) as wpkp,
            tc.tile_pool(name="wspp", bufs=RINGSP) as wspp,
            tc.tile_pool(name="cap", bufs=4) as cap,
            tc.tile_pool(name="cbp", bufs=4) as cbp,
            tc.tile_pool(name="xep", bufs=4) as xep,
            tc.tile_pool(name="xin", bufs=8) as xin,
            tc.tile_pool(name="part", bufs=NGRP) as part,
            tc.tile_pool(name="yout", bufs=2) as yout,
            tc.tile_pool(name="psum", bufs=8, space="PSUM") as psum_pool,
        ):
            bias_sb = const.tile([P, o_sh], f32, tag="bias_sb")
            bias_bf = const.tile([1, o_sh], bf16, tag="bias_bf")
            b_sp = const.tile([1, o_sh], f32, tag="b_sp")
            ones = const.tile([1, P], bf16, tag="ones")
            nc.vector.memset(ones[:], 1.0)
            # K=128 warmup operands: full-array activity so PE_HAM's
            # SHORT window sees real busy-ness and opens the clock gate
            # during warmup (K=1 warmups don't register).
            wl = const.tile([P, P], bf16, tag="wl")
            nc.vector.memset(wl[:], 0.0)
            wr = const.tile([P, o_sh], bf16, tag="wr")
            nc.vector.memset(wr[:], 0.0)

            # Dummy first ACT: hoists the ~1.3us ACT_TABLE_LOAD to the
            # scalar engine's first instruction, off the bias/softplus
            # critical path.
            nc.scalar.activation(b_sp[:, 0:P], ones[:], act_exp)

            # ---- construction helpers
            pks = {}
            sps = {}

            def emit_pk_dma(k, eng):
                pk = wpkp.tile([P, 3 * o_sh], bf16, tag="pk", name=f"pk{k}")
                eng.dma_start(out=pk[:], in_=wpk[k])
                pks[k] = pk

            def emit_sp(k):
                # softplus(rho) = ln(1 + exp(rho)); Exp+Ln share one ACT
                # table set (natural_log_exp_and_others).
                rho_t = pks[k][:, 2 * o_sh : 3 * o_sh].bitcast(f16)
                sp_f = wspp.tile([P, o_sh], f16, tag="spf", name=f"spf{k}")
                sp_t = wspp.tile([P, o_sh], bf16, tag="sp", name=f"sp{k}")
                nc.scalar.activation(sp_f[:], rho_t[:], act_exp)
                nc.scalar.activation(sp_t[:], sp_f[:], act_ln, bias=1.0)
                sps[k] = sp_t

            def emit_wt(k):
                pk = pks[k]
                mu_t = pk[:, 0:o_sh]
                eps_t = pk[:, o_sh : 2 * o_sh]
                sp_t = sps[k]
                nc.vector.tensor_mul(out=sp_t[:], in0=sp_t[:], in1=eps_t[:])
                nc.vector.tensor_add(out=WT_bf[:, k, :], in0=sp_t[:], in1=mu_t[:])

            # pk0-3 lead on the (otherwise idle) scalar queue: weights
            # for the first rows arrive in parallel with the first x
            # chunks on sync.
            for k in range(4):
                emit_pk_dma(k, nc.scalar)

            # First x: two single-k chunks (small => early completion
            # sems), ahead of everything else on sync.
            cak0 = const.tile([P, 1, GROUP, P], bf16, tag="cak0")
            cak1 = const.tile([P, 1, GROUP, P], bf16, tag="cak1")
            nc.sync.dma_start(out=cak0[:], in_=xga[:, 0:1])
            nc.sync.dma_start(out=cak1[:], in_=xga[:, 1:2])

            # Bias inputs (6 KiB, one packed partition-0 [1, 3*o_sh]
            # DMA: mu | rho | eps).
            b_all = const.tile([1, 3 * o_sh], f32, tag="b_all")
            nc.sync.dma_start(out=b_all[:], in_=bpk[:])
            b_mu = b_all[:, 0:o_sh]
            b_rho = b_all[:, o_sh : 2 * o_sh]
            b_eps = b_all[:, 2 * o_sh : 3 * o_sh]
            nc.scalar.activation(b_sp[:], b_rho, act_exp)
            nc.scalar.activation(b_sp[:], b_sp[:], act_ln, bias=1.0)
            nc.vector.tensor_mul(out=b_sp[:], in0=b_sp[:], in1=b_eps)
            nc.vector.tensor_add(out=bias_bf[:], in0=b_sp[:], in1=b_mu)

            # softplus for the scalar-delivered pk0-3 right after the
            # bias ACTs in the scalar program.
            for k in range(4):
                emit_sp(k)

            # PE warmup + bias broadcast emitted early so the DVE-side
            # bias_sb eviction precedes construction in the DVE program.
            warm_ps = psum_pool.tile([P, o_sh], f32, tag="ps", name="warm_ps")
            for w in range(N_WARM):
                nc.tensor.matmul(warm_ps[:], lhsT=wl[:], rhs=wr[:])
            bias_ps = psum_pool.tile([P, o_sh], f32, tag="ps", name="bias_ps")
            nc.tensor.matmul(bias_ps[:], lhsT=ones[:], rhs=bias_bf[:])
            nc.vector.tensor_copy(out=bias_sb[:], in_=bias_ps[:])

            WT_bf = const.tile([P, KB, o_sh], bf16, tag="WT_bf")

            # ---- delivery program (sync queue), consumption order.
            # Ring pools (cap/cbp/xep) put reuse gates on the later x
            # chunks, so the static scheduler cannot hoist them ahead of
            # the pass-A-critical deliveries.
            ca = [cak0, cak1]  # k0, k1 singles
            capair = []
            t = cap.tile([P, 2, GROUP, P], bf16, tag="ca", name="ca1")
            nc.sync.dma_start(out=t[:], in_=xga[:, 2:4])
            capair.append(t)
            for j in range(2, NPAIR):
                emit_pk_dma(2 * j, nc.sync)
                emit_sp(2 * j)
                emit_pk_dma(2 * j + 1, nc.sync)
                emit_sp(2 * j + 1)
                t = cap.tile([P, 2, GROUP, P], bf16, tag="ca", name=f"ca{j}")
                nc.sync.dma_start(out=t[:], in_=xga[:, 2 * j : 2 * j + 2])
                capair.append(t)
            xe0_sb = []
            for e in range(EXTRA):
                t = xep.tile([P, HALF, P], bf16, tag="xe0", name=f"xe0_{e}")
                nc.sync.dma_start(out=t[:], in_=xe0[e])
                xe0_sb.append(t)
            # extras' second halves (whole block: pass B's k16 row reads
            # all extras at once)
            xe1_sb = const.tile([P, HALF, EXTRA, P], bf16, tag="xe1", name="xe1")
            nc.sync.dma_start(out=xe1_sb[:], in_=xe1[:])
            # Second half: [pk, pk, cb_j] pairs for tiles 0..NB1-1.
            cbt = []
            for j in range(NPAIR):
                emit_pk_dma(HALF + 2 * j, nc.sync)
                emit_sp(HALF + 2 * j)
                emit_pk_dma(HALF + 2 * j + 1, nc.sync)
                emit_sp(HALF + 2 * j + 1)
                t = cbp.tile([P, 2, NB1, P], bf16, tag="cb", name=f"cb{j}")
                nc.sync.dma_start(out=t[:], in_=xgb[:, 2 * j : 2 * j + 2])
                cbt.append(t)

            def glhs(i, k):
                """lhsT AP for group tile i (0..NGRP-1), k-block k
                (B2 tiles 2..7 second halves are passed explicitly)."""
                if k < HALF:
                    if i < GROUP:
                        if k < 2:
                            return ca[k][:, 0, i, :]
                        return capair[(k - 2) // 2][:, (k - 2) % 2, i, :]
                    return xe0_sb[i - GROUP][:, k, :]
                if i < NB1:
                    return cbt[(k - HALF) // 2][:, (k - HALF) % 2, i, :]
                assert i >= GROUP
                return xe1_sb[:, k - HALF, i - GROUP, :]

            # ---- DVE construction for the first half (pass-A weights)
            for k in range(HALF):
                emit_wt(k)

            # Pass A: tiles 0-7, k 0..15, k-interleaved.
            pss = [
                psum_pool.tile([P, o_sh], f32, tag="ps", name=f"ps_a{bt}")
                for bt in range(GROUP)
            ]
            for k in range(HALF):
                for i in range(GROUP):
                    nc.tensor.matmul(
                        pss[i][:],
                        lhsT=glhs(i, k),
                        rhs=WT_bf[:, k, :],
                        start=(k == 0),
                        stop=(k == HALF - 1),
                    )
            parts = {}
            for i in range(GROUP):
                pa = part.tile([P, o_sh], bf16, tag="pA", name=f"pA_{i}")
                nc.vector.tensor_add(out=pa[:], in0=pss[i][:], in1=bias_sb[:])
                parts[i] = pa
            # A2: extras' first halves at full speed (W^T 0..HALF
            # resident); second-half constructions interleave on DVE so
            # each partsA2 eviction stays unblocked.
            wt_next = HALF
            for e in range(GROUP, NGRP):
                if wt_next < KB:
                    emit_wt(wt_next)
                    emit_wt(wt_next + 1)
                    wt_next += 2
                ps = psum_pool.tile([P, o_sh], f32, tag="ps", name=f"ps_a{e}")
                for k in range(HALF):
                    nc.tensor.matmul(
                        ps[:],
                        lhsT=glhs(e, k),
                        rhs=WT_bf[:, k, :],
                        start=(k == 0),
                        stop=(k == HALF - 1),
                    )
                pa = part.tile([P, o_sh], bf16, tag="pA", name=f"pA_{e}")
                nc.vector.tensor_add(out=pa[:], in0=ps[:], in1=bias_sb[:])
                parts[e] = pa
            for k in range(wt_next, KB):
                emit_wt(k)

            def split_tail(ps, pa, bt):
                y_sb = yout.tile([P, o_sh], f32, tag="y_sb")
                nc.vector.tensor_add(out=y_sb[:], in0=ps[:], in1=pa[:])
                nc.scalar.dma_start(out=y[bt * P : (bt + 1) * P, :], in_=y_sb[:])

            # B: extras + tiles 0..NB1-1, k 16..31, k-interleaved — the
            # delivery-paced pass rides the pk16-31 + cb chunk stream.
            b_tiles = list(range(GROUP, NGRP)) + list(range(NB1))
            psb = {
                i: psum_pool.tile([P, o_sh], f32, tag="ps", name=f"ps_b{i}")
                for i in b_tiles
            }
            for k in range(HALF, KB):
                for i in b_tiles:
                    nc.tensor.matmul(
                        psb[i][:],
                        lhsT=glhs(i, k),
                        rhs=WT_bf[:, k, :],
                        start=(k == HALF),
                        stop=(k == KB - 1),
                    )
            for i in b_tiles:
                split_tail(psb[i], parts[i], i)
            # B2: tiles NB1..7 second halves at full speed, off halves
            # delivered through the xin ring (its first allocations).
            for i in range(NB1, GROUP):
                hb = xin.tile([P, HALF, P], bf16, tag="xh", name=f"hb{i}")
                nc.sync.dma_start(out=hb[:], in_=xgc[i - NB1])
                ps = psum_pool.tile([P, o_sh], f32, tag="ps", name=f"ps_b{i}")
                for k in range(HALF, KB):
                    nc.tensor.matmul(
                        ps[:],
                        lhsT=hb[:, k - HALF, :],
                        rhs=WT_bf[:, k, :],
                        start=(k == HALF),
                        stop=(k == KB - 1),
                    )
                split_tail(ps, parts[i], i)

            # ---- remaining tiles stream one PSUM bank each; x arrives
            # as two half-tile DMAs through the xin ring.
            for bt in range(NGRP, BT):
                ha = xin.tile([P, HALF, P], bf16, tag="xh")
                hb = xin.tile([P, HALF, P], bf16, tag="xh")
                nc.sync.dma_start(out=ha[:], in_=xs[bt - NGRP, :, :HALF])
                nc.sync.dma_start(out=hb[:], in_=xs[bt - NGRP, :, HALF:])
                ps = psum_pool.tile([P, o_sh], f32, tag="ps")
                for k in range(KB):
                    xh = ha if k < HALF else hb
                    nc.tensor.matmul(
                        ps[:],
                        lhsT=xh[:, k % HALF, :],
                        rhs=WT_bf[:, k, :],
                        start=(k == 0),
                        stop=(k == KB - 1),
                    )
                y_sb = yout.tile([P, o_sh], f32, tag="y_sb")
                nc.vector.tensor_add(out=y_sb[:], in0=ps[:], in1=bias_sb[:])
                nc.scalar.dma_start(out=y[bt * P : (bt + 1) * P, :], in_=y_sb[:])

    # Skip bacc's pre-placed InstLoadActFuncSet: on large graphs walrus's
    # parallel-pass fork can separate the hoisted load from its activations
    # ("No Act func set exist for this instruction"); walrus's own lower_act
    # placement handles forked subgraphs correctly.
    nc.insert_act_table_loads = lambda: None
    nc.compile()
    return nc


def _prep_x(x):
    """[batch, in_f] fp32 -> bf16 tiles with layouts matching the DRAM
    tensors (x_t[..., pi, ..., bi] = x[bt*128 + bi, po*128 + pi]):
      xga [P, HALF, GROUP, P]    tiles 0-7,  k 0..15, k-major
      xgb [P, HALF, NB1, P]      tiles 0-1,  k 16..31, k-major
      xgc [6, P, HALF, P]        tiles 2-7,  k 16..31, tile-major
      xe0 [EXTRA, P, HALF, P]    extras 8-13, k 0..15, tile-major
      xe1 [P, HALF, EXTRA, P]    extras 8-13, k 16..31, k-major
      xs  [BT-NGRP, P, KB, P]    stream tiles, tile-major
    """
    batch, in_f = x.shape
    KB = in_f // P
    HALF = KB // 2
    BT = batch // P
    xbf = x.astype(ml_dtypes.bfloat16)
    xbf = xbf.reshape(BT, P, KB, P)  # [bt, bi, po, pi]
    xt = xbf.transpose(0, 3, 2, 1)  # [bt, pi, po, bi]
    xga = np.ascontiguousarray(xt[:GROUP, :, :HALF].transpose(1, 2, 0, 3))
    xgb = np.ascontiguousarray(xt[:NB1, :, HALF:].transpose(1, 2, 0, 3))
    xgc = np.ascontiguousarray(xt[NB1:GROUP, :, HALF:])
    xe0 = np.ascontiguousarray(xt[GROUP:NGRP, :, :HALF])
    xe1 = np.ascontiguousarray(xt[GROUP:NGRP, :, HALF:].transpose(1, 2, 0, 3))
    xs = np.ascontiguousarray(xt[NGRP:])
    return xga, xgb, xgc, xe0, xe1, xs


def _tile_w(w, dtype):
    """[o_sh, in_f] -> tiled [KB, 128, o_sh] with w_t[k, pi, o] = w[o, k*128 + pi]."""
    o_sh, in_f = w.shape
    return np.ascontiguousarray(w.T.reshape(in_f // P, P, o_sh)).astype(dtype)


def _prep_wpk(wmu, wrho, weps):
    """Pack mu (bf16), eps (bf16), rho (fp16 bits viewed as bf16) into one
    bf16-typed [KB, 128, 3*o_sh] tensor — one DMA per K-block."""
    mu = _tile_w(wmu, ml_dtypes.bfloat16)
    eps = _tile_w(weps, ml_dtypes.bfloat16)
    rho = _tile_w(wrho, np.float16).view(ml_dtypes.bfloat16)
    return np.ascontiguousarray(np.concatenate([mu, eps, rho], axis=2))


def make_in_maps(x, weight_mu, weight_rho, bias_mu, bias_rho, weight_eps, bias_eps):
    o_sh = OUT_F // N_CORES
    xga, xgb, xgc, xe0, xe1, xs = _prep_x(np.asarray(x, dtype=np.float32))
    wmu = np.asarray(weight_mu, dtype=np.float32)
    wrho = np.asarray(weight_rho, dtype=np.float32)
    weps = np.asarray(weight_eps, dtype=np.float32)
    bpk = np.stack(
        [
            np.asarray(bias_mu, dtype=np.float32),
            np.asarray(bias_rho, dtype=np.float32),
            np.asarray(bias_eps, dtype=np.float32),
        ]
    )  # [3, OUT_F]

    in_maps = []
    for c in range(N_CORES):
        rs = slice(c * o_sh, (c + 1) * o_sh)
        in_maps.append(
            {
                "x_ga": xga,
                "x_gb": xgb,
                "x_gc": xgc,
                "x_e0": xe0,
                "x_e1": xe1,
                "x_s": xs,
                "wpk_t": _prep_wpk(wmu[rs], wrho[rs], weps[rs]),
                "bias_pk": np.ascontiguousarray(bpk[:, rs].reshape(1, -1)),
            }
        )
    return in_maps


def kernel(x, weight_mu, weight_rho, bias_mu, bias_rho, weight_eps, bias_eps):
    o_sh = OUT_F // N_CORES
    key = (x.shape, o_sh)
    if key not in _NC_CACHE:
        _NC_CACHE[key] = build_nc(x.shape[0], x.shape[1], o_sh)
    nc = _NC_CACHE[key]

    in_maps = make_in_maps(
        x, weight_mu, weight_rho, bias_mu, bias_rho, weight_eps, bias_eps
    )
    res = run_bass_kernel_spmd(nc, in_maps, core_ids=list(range(N_CORES)))
    return np.concatenate([res.results[c]["y"] for c in range(N_CORES)], axis=1)


# revision 20
# speedup vs baseline: 1.0454x; 1.0020x over previous
"""BayesianLinear (y = x @ (mu + softplus(rho) * eps).T + bias) on 8 TRN2 cores.

Column-parallel sharding: each core owns OUT_F/8 = 512 output features.

Host-side prep is pure layout/precision staging (no reference math):
  - x is cast to bf16 and pre-tiled into the SBUF layout the TensorEngine
    needs for its stationary operand, in delivery-order groupings:
    k-pair chunks across tiles (ca: tiles 0-7 first half, cb: tiles 0-1
    second half), per-tile halves (extras 8-13 first halves, tiles 2-7
    second halves, stream tiles), k-major block for extras' second
    halves.
  - weight mu/rho/eps shards are transposed to [in_f, o_sh] and packed
    per single K-block into one bf16-typed tensor (mu bf16 | eps bf16 |
    rho fp16-bits). rho ships fp16 because softplus amplifies its
    quantization ~3x.

Device per core (phase 1 sized so PE work >= delivery time at the
~290 GB/s the HBM actually delivers: 14 parked tiles x 6.9us = 97us of
PE work vs 27.7 MB of phase-1 bytes = 96us):
  1. Delivery: sync queue carries [bias, ca0, ca1, then per k-pair
     pk,pk,chunk in consumption order, extras/B2 halves, stream tiles];
     the scalar queue carries ONLY pk0-3 (a parallel fast start while
     it has nothing else — its first ACT can't run before the ~2.7us
     table load anyway) and later the y writes, whose eviction-wait
     must not head-of-line block x prefetch. A single main queue
     avoids SDMA round-robin unfairness (measured: big chunks on a
     second queue starve 393KB pk singles to ~60GB/s).
  2. W^T construction per K-block: Exp then Ln(1+x) on ACT (one shared
     table set), mul(eps)/add(mu) on DVE at 2x bf16 rate into the
     resident W^T tile [128, 32, 512]. The DVE program interleaves
     construction with partial evictions in consumption order so PSUM
     banks free on time.
  3. PE program: K=128 dummy warmups (real array activity, so the HAM
     clock-gate opens during warmup — K=1 warmups don't register and
     the first ~4us of real matmuls would run at 1.2 GHz), bias
     broadcast, then split-K over 14 tiles with 8 PSUM banks:
       A:  tiles 0-7, k 0-15, k-interleaved, delivery-paced with
           per-k-pair micro-stalls << 3.4us (so the HAM never
           re-throttles), bias-added partials parked in SBUF bf16
       A2: extras 8-13, k 0-15, full speed off resident W^T
       B:  extras 8-13 + tiles 0-1, k 16-31, k-interleaved riding the
           pk16-31 stream
       B2: tiles 2-7, k 16-31, full speed off late-arriving halves
     Remaining 50 tiles stream one PSUM bank each; DVE fuses eviction
     with the partial/bias add.
"""

import numpy as np
import ml_dtypes

import concourse.bacc as bacc
import concourse.mybir as mybir
import concourse.tile as tile
from concourse.bass_utils import run_bass_kernel_spmd

BATCH = 8192
IN_F = 4096
OUT_F = 4096
N_CORES = 8
P = 128

GROUP = 8
EXTRA = 6
NGRP = GROUP + EXTRA
NB1 = 2  # tiles 0..NB1-1 join the interleaved pass B; tiles NB1..7 are B2

_NC_CACHE = {}


def build_nc(batch=BATCH, in_f=IN_F, o_sh=OUT_F // N_CORES):
    KB = in_f // P  # K-blocks of 128 along the contraction dim
    BT = batch // P  # 128-row output tiles
    HALF = KB // 2
    NPAIR = HALF // 2  # k-pairs per half

    nc = bacc.Bacc(
        "TRN2",
        target_bir_lowering=False,
        debug=False,
        enable_asserts=False,
        num_devices=N_CORES,
    )
    bf16 = mybir.dt.bfloat16
    f16 = mybir.dt.float16
    f32 = mybir.dt.float32

    # x DRAM tensors, one per delivery grouping
    xga = nc.declare_dram_parameter("x_ga", [P, HALF, GROUP, P], bf16, isOutput=False)
    xgb = nc.declare_dram_parameter("x_gb", [P, HALF, NB1, P], bf16, isOutput=False)
    xgc = nc.declare_dram_parameter(
        "x_gc", [GROUP - NB1, P, HALF, P], bf16, isOutput=False
    )
    xe0 = nc.declare_dram_parameter("x_e0", [EXTRA, P, HALF, P], bf16, isOutput=False)
    xe1 = nc.declare_dram_parameter("x_e1", [P, HALF, EXTRA, P], bf16, isOutput=False)
    xs = nc.declare_dram_parameter("x_s", [BT - NGRP, P, KB, P], bf16, isOutput=False)
    wpk = nc.declare_dram_parameter("wpk_t", [KB, P, 3 * o_sh], bf16, isOutput=False)
    bpk = nc.declare_dram_parameter("bias_pk", [1, 3 * o_sh], f32, isOutput=False)
    y = nc.declare_dram_parameter("y", [batch, o_sh], f32, isOutput=True)

    act_exp = mybir.ActivationFunctionType.Exp
    act_ln = mybir.ActivationFunctionType.Ln

    N_WARM = 20
    # pk ring deep enough that its trigger gates (wait wt_{k-RINGPK})
    # never bind at runtime — a gated trigger blocks the whole HWDGE
    # queue head behind it. sp ring shallow (construction-local).
    RINGPK = 6
    RINGSP = 3

    with tile.TileContext(nc) as tc:
        with (
            tc.tile_pool(name="const", bufs=1) as const,
            tc.tile_pool(name="wpkp", bufs=RINGPK) as wpkp,
            tc.tile_pool(name="wspp", bufs=RINGSP) as wspp,
            tc.tile_pool(name="cap", bufs=4) as cap,
            tc.tile_pool(name="cbp", bufs=4) as cbp,
            tc.tile_pool(name="xep", bufs=4) as xep,
            tc.tile_pool(name="xin", bufs=8) as xin,
            tc.tile_pool(name="part", bufs=NGRP) as part,
            tc.tile_pool(name="yout", bufs=2) as yout,
            tc.tile_pool(name="psum", bufs=8, space="PSUM") as psum_pool,
        ):
            bias_sb = const.tile([P, o_sh], f32, tag="bias_sb")
            bias_bf = const.tile([1, o_sh], bf16, tag="bias_bf")
            b_sp = const.tile([1, o_sh], f32, tag="b_sp")
            ones = const.tile([1, P], bf16, tag="ones")
            nc.vector.memset(ones[:], 1.0)
            # K=128 warmup operands: full-array activity so PE_HAM's
            # SHORT window sees real busy-ness and opens the clock gate
            # during warmup (K=1 warmups don't register).
            wl = const.tile([P, P], bf16, tag="wl")
            nc.vector.memset(wl[:], 0.0)
            wr = const.tile([P, o_sh], bf16, tag="wr")
            nc.vector.memset(wr[:], 0.0)

            # Dummy first ACT: hoists the ~1.3us ACT_TABLE_LOAD to the
            # scalar engine's first instruction, off the bias/softplus
            # critical path.
            nc.scalar.activation(b_sp[:, 0:P], ones[:], act_exp)

            # ---- construction helpers
            pks = {}
            sps = {}

            def emit_pk_dma(k, eng):
                pk = wpkp.tile([P, 3 * o_sh], bf16, tag="pk", name=f"pk{k}")
                eng.dma_start(out=pk[:], in_=wpk[k])
                pks[k] = pk

            def emit_sp(k):
                # softplus(rho) = ln(1 + exp(rho)); Exp+Ln share one ACT
                # table set (natural_log_exp_and_others).
                rho_t = pks[k][:, 2 * o_sh : 3 * o_sh].bitcast(f16)
                sp_f = wspp.tile([P, o_sh], f16, tag="spf", name=f"spf{k}")
                sp_t = wspp.tile([P, o_sh], bf16, tag="sp", name=f"sp{k}")
                nc.scalar.activation(sp_f[:], rho_t[:], act_exp)
                nc.scalar.activation(sp_t[:], sp_f[:], act_ln, bias=1.0)
                sps[k] = sp_t

            def emit_wt(k):
                pk = pks[k]
                mu_t = pk[:, 0:o_sh]
                eps_t = pk[:, o_sh : 2 * o_sh]
                sp_t = sps[k]
                nc.vector.tensor_mul(out=sp_t[:], in0=sp_t[:], in1=eps_t[:])
                nc.vector.tensor_add(out=WT_bf[:, k, :], in0=sp_t[:], in1=mu_t[:])

            # The SDMA arbiter services qScalarDynamicHW ahead of
            # qSyncDynamicHW (measured across three schedules: whatever
            # sits on the scalar queue starves the sync queue). So the
            # pass-A-critical first x chunks ride the scalar queue —
            # which is otherwise empty until the y writes — and drain
            # in ~3us; everything else lines up on sync in consumption
            # order and gets full bandwidth once they're through.
            cak0 = const.tile([P, 1, GROUP, P], bf16, tag="cak0")
            cak1 = const.tile([P, 1, GROUP, P], bf16, tag="cak1")
            nc.scalar.dma_start(out=cak0[:], in_=xga[:, 0:1])
            nc.scalar.dma_start(out=cak1[:], in_=xga[:, 1:2])

            # Bias inputs (6 KiB, one packed partition-0 [1, 3*o_sh]
            # DMA: mu | rho | eps) lead the sync queue, then pk0-3.
            b_all = const.tile([1, 3 * o_sh], f32, tag="b_all")
            nc.sync.dma_start(out=b_all[:], in_=bpk[:])
            for k in range(4):
                emit_pk_dma(k, nc.sync)
            b_mu = b_all[:, 0:o_sh]
            b_rho = b_all[:, o_sh : 2 * o_sh]
            b_eps = b_all[:, 2 * o_sh : 3 * o_sh]
            nc.scalar.activation(b_sp[:], b_rho, act_exp)
            nc.scalar.activation(b_sp[:], b_sp[:], act_ln, bias=1.0)
            nc.vector.tensor_mul(out=b_sp[:], in0=b_sp[:], in1=b_eps)
            nc.vector.tensor_add(out=bias_bf[:], in0=b_sp[:], in1=b_mu)

            # softplus for the scalar-delivered pk0-3 right after the
            # bias ACTs in the scalar program.
            for k in range(4):
                emit_sp(k)

            # PE warmup + bias broadcast emitted early so the DVE-side
            # bias_sb eviction precedes construction in the DVE program.
            warm_ps = psum_pool.tile([P, o_sh], f32, tag="ps", name="warm_ps")
            for w in range(N_WARM):
                nc.tensor.matmul(warm_ps[:], lhsT=wl[:], rhs=wr[:])
            bias_ps = psum_pool.tile([P, o_sh], f32, tag="ps", name="bias_ps")
            nc.tensor.matmul(bias_ps[:], lhsT=ones[:], rhs=bias_bf[:])
            nc.vector.tensor_copy(out=bias_sb[:], in_=bias_ps[:])

            WT_bf = const.tile([P, KB, o_sh], bf16, tag="WT_bf")

            # ---- delivery program (sync queue), consumption order.
            # Ring pools (cap/cbp/xep) put reuse gates on the later x
            # chunks, so the static scheduler cannot hoist them ahead of
            # the pass-A-critical deliveries.
            ca = [cak0, cak1]  # k0, k1 singles
            capair = []
            t = cap.tile([P, 2, GROUP, P], bf16, tag="ca", name="ca1")
            nc.scalar.dma_start(out=t[:], in_=xga[:, 2:4])
            capair.append(t)
            for j in range(2, NPAIR):
                emit_pk_dma(2 * j, nc.sync)
                emit_sp(2 * j)
                emit_pk_dma(2 * j + 1, nc.sync)
                emit_sp(2 * j + 1)
                t = cap.tile([P, 2, GROUP, P], bf16, tag="ca", name=f"ca{j}")
                nc.sync.dma_start(out=t[:], in_=xga[:, 2 * j : 2 * j + 2])
                capair.append(t)
            xe0_sb = []
            for e in range(EXTRA):
                t = xep.tile([P, HALF, P], bf16, tag="xe0", name=f"xe0_{e}")
                nc.sync.dma_start(out=t[:], in_=xe0[e])
                xe0_sb.append(t)
            # extras' second halves (whole block: pass B's k16 row reads
            # all extras at once)
            xe1_sb = const.tile([P, HALF, EXTRA, P], bf16, tag="xe1", name="xe1")
            nc.sync.dma_start(out=xe1_sb[:], in_=xe1[:])
            # Second half: [pk, pk, cb_j] pairs for tiles 0..NB1-1.
            cbt = []
            for j in range(NPAIR):
                emit_pk_dma(HALF + 2 * j, nc.sync)
                emit_sp(HALF + 2 * j)
                emit_pk_dma(HALF + 2 * j + 1, nc.sync)
                emit_sp(HALF + 2 * j + 1)
                t = cbp.tile([P, 2, NB1, P], bf16, tag="cb", name=f"cb{j}")
                nc.sync.dma_start(out=t[:], in_=xgb[:, 2 * j : 2 * j + 2])
                cbt.append(t)

            def glhs(i, k):
                """lhsT AP for group tile i (0..NGRP-1), k-block k
                (B2 tiles 2..7 second halves are passed explicitly)."""
                if k < HALF:
                    if i < GROUP:
                        if k < 2:
                            return ca[k][:, 0, i, :]
                        return capair[(k - 2) // 2][:, (k - 2) % 2, i, :]
                    return xe0_sb[i - GROUP][:, k, :]
                if i < NB1:
                    return cbt[(k - HALF) // 2][:, (k - HALF) % 2, i, :]
                assert i >= GROUP
                return xe1_sb[:, k - HALF, i - GROUP, :]

            # ---- DVE construction for the first half (pass-A weights)
            for k in range(HALF):
                emit_wt(k)

            # Pass A: tiles 0-7, k 0..15, k-interleaved.
            pss = [
                psum_pool.tile([P, o_sh], f32, tag="ps", name=f"ps_a{bt}")
                for bt in range(GROUP)
            ]
            for k in range(HALF):
                for i in range(GROUP):
                    nc.tensor.matmul(
                        pss[i][:],
                        lhsT=glhs(i, k),
                        rhs=WT_bf[:, k, :],
                        start=(k == 0),
                        stop=(k == HALF - 1),
                    )
            parts = {}
            for i in range(GROUP):
                pa = part.tile([P, o_sh], bf16, tag="pA", name=f"pA_{i}")
                nc.vector.tensor_add(out=pa[:], in0=pss[i][:], in1=bias_sb[:])
                parts[i] = pa
            # A2: extras' first halves at full speed (W^T 0..HALF
            # resident); second-half constructions interleave on DVE so
            # each partsA2 eviction stays unblocked.
            wt_next = HALF
            for e in range(GROUP, NGRP):
                if wt_next < KB:
                    emit_wt(wt_next)
                    emit_wt(wt_next + 1)
                    wt_next += 2
                ps = psum_pool.tile([P, o_sh], f32, tag="ps", name=f"ps_a{e}")
                for k in range(HALF):
                    nc.tensor.matmul(
                        ps[:],
                        lhsT=glhs(e, k),
                        rhs=WT_bf[:, k, :],
                        start=(k == 0),
                        stop=(k == HALF - 1),
                    )
                pa = part.tile([P, o_sh], bf16, tag="pA", name=f"pA_{e}")
                nc.vector.tensor_add(out=pa[:], in0=ps[:], in1=bias_sb[:])
                parts[e] = pa
            for k in range(wt_next, KB):
                emit_wt(k)

            def split_tail(ps, pa, bt):
                y_sb = yout.tile([P, o_sh], f32, tag="y_sb")
                nc.vector.tensor_add(out=y_sb[:], in0=ps[:], in1=pa[:])
                nc.scalar.dma_start(out=y[bt * P : (bt + 1) * P, :], in_=y_sb[:])

            # B: extras + tiles 0..NB1-1, k 16..31, k-interleaved — the
            # delivery-paced pass rides the pk16-31 + cb chunk stream.
            b_tiles = list(range(GROUP, NGRP)) + list(range(NB1))
            psb = {
                i: psum_pool.tile([P, o_sh], f32, tag="ps", name=f"ps_b{i}")
                for i in b_tiles
            }
            for k in range(HALF, KB):
                for i in b_tiles:
                    nc.tensor.matmul(
                        psb[i][:],
                        lhsT=glhs(i, k),
                        rhs=WT_bf[:, k, :],
                        start=(k == HALF),
                        stop=(k == KB - 1),
                    )
            for i in b_tiles:
                split_tail(psb[i], parts[i], i)
            # B2: tiles NB1..7 second halves at full speed, off halves
            # delivered through the xin ring (its first allocations).
            for i in range(NB1, GROUP):
                hb = xin.tile([P, HALF, P], bf16, tag="xh", name=f"hb{i}")
                nc.sync.dma_start(out=hb[:], in_=xgc[i - NB1])
                ps = psum_pool.tile([P, o_sh], f32, tag="ps", name=f"ps_b{i}")
                for k in range(HALF, KB):
                    nc.tensor.matmul(
                        ps[:],
                        lhsT=hb[:, k - HALF, :],
                        rhs=WT_bf[:, k, :],
                        start=(k == HALF),
                        stop=(k == KB - 1),
                    )
                split_tail(ps, parts[i], i)

            # ---- remaining tiles stream one PSUM bank each; x arrives
            # as two half-tile DMAs through the xin ring.
            for bt in range(NGRP, BT):
                ha = xin.tile([P, HALF, P], bf16, tag="xh")
                hb = xin.tile([P, HALF, P], bf16, tag="xh")
                nc.sync.dma_start(out=ha[:], in_=xs[bt - NGRP, :, :HALF])
                nc.sync.dma_start(out=hb[:], in_=xs[bt - NGRP, :, HALF:])
                ps = psum_pool.tile([P, o_sh], f32, tag="ps")
                for k in range(KB):
                    xh = ha if k < HALF else hb
                    nc.tensor.matmul(
                        ps[:],
                        lhsT=xh[:, k % HALF, :],
                        rhs=WT_bf[:, k, :],
                        start=(k == 0),
                        stop=(k == KB - 1),
                    )
                y_sb = yout.tile([P, o_sh], f32, tag="y_sb")
                nc.vector.tensor_add(out=y_sb[:], in0=ps[:], in1=bias_sb[:])
                nc.scalar.dma_start(out=y[bt * P : (bt + 1) * P, :], in_=y_sb[:])

    # Skip bacc's pre-placed InstLoadActFuncSet: on large graphs walrus's
    # parallel-pass fork can separate the hoisted load from its activations
    # ("No Act func set exist for this instruction"); walrus's own lower_act
    # placement handles forked subgraphs correctly.
    nc.insert_act_table_loads = lambda: None
    nc.compile()
    return nc


def _prep_x(x):
    """[batch, in_f] fp32 -> bf16 tiles with layouts matching the DRAM
    tensors (x_t[..., pi, ..., bi] = x[bt*128 + bi, po*128 + pi]):
      xga [P, HALF, GROUP, P]    tiles 0-7,  k 0..15, k-major
      xgb [P, HALF, NB1, P]      tiles 0-1,  k 16..31, k-major
      xgc [6, P, HALF, P]        tiles 2-7,  k 16..31, tile-major
      xe0 [EXTRA, P, HALF, P]    extras 8-13, k 0..15, tile-major
      xe1 [P, HALF, EXTRA, P]    extras 8-13, k 16..31, k-major
      xs  [BT-NGRP, P, KB, P]    stream tiles, tile-major
    """
    batch, in_f = x.shape
    KB = in_f // P
    HALF = KB // 2
    BT = batch // P
    xbf = x.astype(ml_dtypes.bfloat16)
    xbf = xbf.reshape(BT, P, KB, P)  # [bt, bi, po, pi]
    xt = xbf.transpose(0, 3, 2, 1)  # [bt, pi, po, bi]
    xga = np.ascontiguousarray(xt[:GROUP, :, :HALF].transpose(1, 2, 0, 3))
    xgb = np.ascontiguousarray(xt[:NB1, :, HALF:].transpose(1, 2, 0, 3))
    xgc = np.ascontiguousarray(xt[NB1:GROUP, :, HALF:])
    xe0 = np.ascontiguousarray(xt[GROUP:NGRP, :, :HALF])
    xe1 = np.ascontiguousarray(xt[GROUP:NGRP, :, HALF:].transpose(1, 2, 0, 3))
    xs = np.ascontiguousarray(xt[NGRP:])
    return xga, xgb, xgc, xe0, xe1, xs


def _tile_w(w, dtype):
    """[o_sh, in_f] -> tiled [KB, 128, o_sh] with w_t[k, pi, o] = w[o, k*128 + pi]."""
    o_sh, in_f = w.shape
    return np.ascontiguousarray(w.T.reshape(in_f // P, P, o_sh)).astype(dtype)


def _prep_wpk(wmu, wrho, weps):
    """Pack mu (bf16), eps (bf16), rho (fp16 bits viewed as bf16) into one
    bf16-typed [KB, 128, 3*o_sh] tensor — one DMA per K-block."""
    mu = _tile_w(wmu, ml_dtypes.bfloat16)
    eps = _tile_w(weps, ml_dtypes.bfloat16)
    rho = _tile_w(wrho, np.float16).view(ml_dtypes.bfloat16)
    return np.ascontiguousarray(np.concatenate([mu, eps, rho], axis=2))


def make_in_maps(x, weight_mu, weight_rho, bias_mu, bias_rho, weight_eps, bias_eps):
    o_sh = OUT_F // N_CORES
    xga, xgb, xgc, xe0, xe1, xs = _prep_x(np.asarray(x, dtype=np.float32))
    wmu = np.asarray(weight_mu, dtype=np.float32)
    wrho = np.asarray(weight_rho, dtype=np.float32)
    weps = np.asarray(weight_eps, dtype=np.float32)
    bpk = np.stack(
        [
            np.asarray(bias_mu, dtype=np.float32),
            np.asarray(bias_rho, dtype=np.float32),
            np.asarray(bias_eps, dtype=np.float32),
        ]
    )  # [3, OUT_F]

    in_maps = []
    for c in range(N_CORES):
        rs = slice(c * o_sh, (c + 1) * o_sh)
        in_maps.append(
            {
                "x_ga": xga,
                "x_gb": xgb,
                "x_gc": xgc,
                "x_e0": xe0,
                "x_e1": xe1,
                "x_s": xs,
                "wpk_t": _prep_wpk(wmu[rs], wrho[rs], weps[rs]),
                "bias_pk": np.ascontiguousarray(bpk[:, rs].reshape(1, -1)),
            }
        )
    return in_maps


def kernel(x, weight_mu, weight_rho, bias_mu, bias_rho, weight_eps, bias_eps):
    o_sh = OUT_F // N_CORES
    key = (x.shape, o_sh)
    if key not in _NC_CACHE:
        _NC_CACHE[key] = build_nc(x.shape[0], x.shape[1], o_sh)
    nc = _NC_CACHE[key]

    in_maps = make_in_maps(
        x, weight_mu, weight_rho, bias_mu, bias_rho, weight_eps, bias_eps
    )
    res = run_bass_kernel_spmd(nc, in_maps, core_ids=list(range(N_CORES)))
    return np.concatenate([res.results[c]["y"] for c in range(N_CORES)], axis=1)


# revision 26
# speedup vs baseline: 1.0530x; 1.0073x over previous
"""BayesianLinear (y = x @ (mu + softplus(rho) * eps).T + bias) on 8 TRN2 cores.

Column-parallel sharding: each core owns OUT_F/8 = 512 output features.

Host-side prep is pure layout/precision staging (no reference math):
  - x is cast to bf16 and pre-tiled into the SBUF layout the TensorEngine
    needs for its stationary operand, in delivery-order groupings:
    k-pair chunks across tiles (ca: tiles 0-7 first half, cb: tiles 0-1
    second half), per-tile halves (extras 8-13 first halves, tiles 2-7
    second halves, stream tiles), k-major block for extras' second
    halves.
  - weight mu/rho/eps shards are transposed to [in_f, o_sh] and packed
    per single K-block into one bf16-typed tensor (mu bf16 | eps bf16 |
    rho fp16-bits). rho ships fp16 because softplus amplifies its
    quantization ~3x.

Device per core (phase 1 sized so PE work >= delivery time at the
~290 GB/s the HBM actually delivers: 14 parked tiles x 6.9us = 97us of
PE work vs 27.7 MB of phase-1 bytes = 96us):
  1. Delivery: sync queue carries [bias, ca0, ca1, then per k-pair
     pk,pk,chunk in consumption order, extras/B2 halves, stream tiles];
     the scalar queue carries ONLY pk0-3 (a parallel fast start while
     it has nothing else — its first ACT can't run before the ~2.7us
     table load anyway) and later the y writes, whose eviction-wait
     must not head-of-line block x prefetch. A single main queue
     avoids SDMA round-robin unfairness (measured: big chunks on a
     second queue starve 393KB pk singles to ~60GB/s).
  2. W^T construction per K-block: Exp then Ln(1+x) on ACT (one shared
     table set), mul(eps)/add(mu) on DVE at 2x bf16 rate into the
     resident W^T tile [128, 32, 512]. The DVE program interleaves
     construction with partial evictions in consumption order so PSUM
     banks free on time.
  3. PE program: K=128 dummy warmups (real array activity, so the HAM
     clock-gate opens during warmup — K=1 warmups don't register and
     the first ~4us of real matmuls would run at 1.2 GHz), bias
     broadcast, then split-K over 14 tiles with 8 PSUM banks:
       A:  tiles 0-7, k 0-15, k-interleaved, delivery-paced with
           per-k-pair micro-stalls << 3.4us (so the HAM never
           re-throttles), bias-added partials parked in SBUF bf16
       A2: extras 8-13, k 0-15, full speed off resident W^T
       B:  extras 8-13 + tiles 0-1, k 16-31, k-interleaved riding the
           pk16-31 stream
       B2: tiles 2-7, k 16-31, full speed off late-arriving halves
     Remaining 50 tiles stream one PSUM bank each; DVE fuses eviction
     with the partial/bias add.
"""

import numpy as np
import ml_dtypes

import concourse.bacc as bacc
import concourse.mybir as mybir
import concourse.tile as tile
from concourse.bass_utils import run_bass_kernel_spmd

BATCH = 8192
IN_F = 4096
OUT_F = 4096
N_CORES = 8
P = 128

GROUP = 8
EXTRA = 6
NGRP = GROUP + EXTRA
NB1 = 2  # tiles 0..NB1-1 join the interleaved pass B; tiles NB1..7 are B2

_NC_CACHE = {}


def build_nc(batch=BATCH, in_f=IN_F, o_sh=OUT_F // N_CORES):
    KB = in_f // P  # K-blocks of 128 along the contraction dim
    BT = batch // P  # 128-row output tiles
    HALF = KB // 2
    NPAIR = HALF // 2  # k-pairs per half

    nc = bacc.Bacc(
        "TRN2",
        target_bir_lowering=False,
        debug=False,
        enable_asserts=False,
        num_devices=N_CORES,
    )
    bf16 = mybir.dt.bfloat16
    f16 = mybir.dt.float16
    f32 = mybir.dt.float32

    # x DRAM tensors, one per delivery grouping
    xga = nc.declare_dram_parameter("x_ga", [P, HALF, GROUP, P], bf16, isOutput=False)
    xgb = nc.declare_dram_parameter("x_gb", [P, HALF, NB1, P], bf16, isOutput=False)
    xgc = nc.declare_dram_parameter(
        "x_gc", [GROUP - NB1, P, HALF, P], bf16, isOutput=False
    )
    xe0 = nc.declare_dram_parameter("x_e0", [EXTRA, P, HALF, P], bf16, isOutput=False)
    xe1 = nc.declare_dram_parameter("x_e1", [P, HALF, EXTRA, P], bf16, isOutput=False)
    xs = nc.declare_dram_parameter("x_s", [BT - NGRP, P, KB, P], bf16, isOutput=False)
    wpk = nc.declare_dram_parameter("wpk_t", [KB, P, 3 * o_sh], bf16, isOutput=False)
    bpk = nc.declare_dram_parameter("bias_pk", [1, 3 * o_sh], f32, isOutput=False)
    y = nc.declare_dram_parameter("y", [batch, o_sh], f32, isOutput=True)

    act_exp = mybir.ActivationFunctionType.Exp
    act_ln = mybir.ActivationFunctionType.Ln

    N_WARM = 20
    # pk ring deep enough that its trigger gates (wait wt_{k-RINGPK})
    # never bind at runtime — a gated trigger blocks the whole HWDGE
    # queue head behind it. sp ring shallow (construction-local).
    RINGPK = 6
    RINGSP = 3

    with tile.TileContext(nc) as tc:
        with (
            tc.tile_pool(name="const", bufs=1) as const,
            tc.tile_pool(name="wpkp", bufs=RINGPK) as wpkp,
            tc.tile_pool(name="wspp", bufs=RINGSP) as wspp,
            tc.tile_pool(name="cap", bufs=4) as cap,
            tc.tile_pool(name="cbp", bufs=4) as cbp,
            tc.tile_pool(name="xep", bufs=4) as xep,
            tc.tile_pool(name="xin", bufs=8) as xin,
            tc.tile_pool(name="part", bufs=NGRP) as part,
            tc.tile_pool(name="yout", bufs=2) as yout,
            tc.tile_pool(name="psum", bufs=8, space="PSUM") as psum_pool,
        ):
            bias_sb = const.tile([P, o_sh], f32, tag="bias_sb")
            bias_bf = const.tile([1, o_sh], bf16, tag="bias_bf")
            b_sp = const.tile([1, o_sh], f32, tag="b_sp")
            ones = const.tile([1, P], bf16, tag="ones")
            nc.vector.memset(ones[:], 1.0)
            # K=128 warmup operands: full-array activity so PE_HAM's
            # SHORT window sees real busy-ness and opens the clock gate
            # during warmup (K=1 warmups don't register).
            wl = const.tile([P, P], bf16, tag="wl")
            nc.vector.memset(wl[:], 0.0)
            wr = const.tile([P, o_sh], bf16, tag="wr")
            nc.vector.memset(wr[:], 0.0)

            # Dummy first ACT: hoists the ~1.3us ACT_TABLE_LOAD to the
            # scalar engine's first instruction, off the bias/softplus
            # critical path.
            nc.scalar.activation(b_sp[:, 0:P], ones[:], act_exp)

            # ---- construction helpers
            pks = {}
            sps = {}

            def emit_pk_dma(k, eng):
                pk = wpkp.tile([P, 3 * o_sh], bf16, tag="pk", name=f"pk{k}")
                eng.dma_start(out=pk[:], in_=wpk[k])
                pks[k] = pk

            def emit_sp(k):
                # softplus(rho) = ln(1 + exp(rho)); Exp+Ln share one ACT
                # table set (natural_log_exp_and_others).
                rho_t = pks[k][:, 2 * o_sh : 3 * o_sh].bitcast(f16)
                sp_f = wspp.tile([P, o_sh], f16, tag="spf", name=f"spf{k}")
                sp_t = wspp.tile([P, o_sh], bf16, tag="sp", name=f"sp{k}")
                nc.scalar.activation(sp_f[:], rho_t[:], act_exp)
                nc.scalar.activation(sp_t[:], sp_f[:], act_ln, bias=1.0)
                sps[k] = sp_t

            def emit_wt(k):
                pk = pks[k]
                mu_t = pk[:, 0:o_sh]
                eps_t = pk[:, o_sh : 2 * o_sh]
                sp_t = sps[k]
                nc.vector.tensor_mul(out=sp_t[:], in0=sp_t[:], in1=eps_t[:])
                nc.vector.tensor_add(out=WT_bf[:, k, :], in0=sp_t[:], in1=mu_t[:])

            # The SDMA arbiter services qScalarDynamicHW ahead of
            # qSyncDynamicHW (measured across three schedules: whatever
            # sits on the scalar queue starves the sync queue). So the
            # pass-A-critical first x chunks ride the scalar queue —
            # which is otherwise empty until the y writes — and drain
            # in ~3us; everything else lines up on sync in consumption
            # order and gets full bandwidth once they're through.
            # Minimal priority-queue head: pk0 (longest chain) then the
            # first two x singles. More q10 triggers would tax the
            # scalar engine and push the softplus stream late.
            cak0 = const.tile([P, 1, GROUP, P], bf16, tag="cak0")
            cak1 = const.tile([P, 1, GROUP, P], bf16, tag="cak1")
            emit_pk_dma(0, nc.scalar)
            nc.scalar.dma_start(out=cak0[:], in_=xga[:, 0:1])
            nc.scalar.dma_start(out=cak1[:], in_=xga[:, 1:2])

            # Bias inputs (6 KiB, one packed partition-0 [1, 3*o_sh]
            # DMA: mu | rho | eps) lead the sync queue, then pk1-3.
            b_all = const.tile([1, 3 * o_sh], f32, tag="b_all")
            nc.sync.dma_start(out=b_all[:], in_=bpk[:])
            for k in range(1, 4):
                emit_pk_dma(k, nc.sync)

            # Bias ACTs right behind the three q10 triggers on the
            # scalar engine (b_all lands before the triggers finish),
            # then the first softplus.
            nc.scalar.activation(b_sp[:], b_all[:, o_sh : 2 * o_sh], act_exp)
            nc.scalar.activation(b_sp[:], b_sp[:], act_ln, bias=1.0)
            nc.vector.tensor_mul(
                out=b_sp[:], in0=b_sp[:], in1=b_all[:, 2 * o_sh : 3 * o_sh]
            )
            nc.vector.tensor_add(out=bias_bf[:], in0=b_sp[:], in1=b_all[:, 0:o_sh])
            for k in range(4):
                emit_sp(k)

            # PE warmup + bias broadcast emitted early so the DVE-side
            # bias_sb eviction precedes construction in the DVE program.
            warm_ps = psum_pool.tile([P, o_sh], f32, tag="ps", name="warm_ps")
            for w in range(N_WARM):
                nc.tensor.matmul(warm_ps[:], lhsT=wl[:], rhs=wr[:])
            bias_ps = psum_pool.tile([P, o_sh], f32, tag="ps", name="bias_ps")
            nc.tensor.matmul(bias_ps[:], lhsT=ones[:], rhs=bias_bf[:])
            nc.vector.tensor_copy(out=bias_sb[:], in_=bias_ps[:])

            WT_bf = const.tile([P, KB, o_sh], bf16, tag="WT_bf")

            # ---- delivery program (sync queue), consumption order.
            # Ring pools (cap/cbp/xep) put reuse gates on the later x
            # chunks, so the static scheduler cannot hoist them ahead of
            # the pass-A-critical deliveries.
            ca = [cak0, cak1]  # k0, k1 singles
            capair = []
            for j in (1, 2):
                t = cap.tile([P, 2, GROUP, P], bf16, tag="ca", name=f"ca{j}")
                nc.sync.dma_start(out=t[:], in_=xga[:, 2 * j : 2 * j + 2])
                capair.append(t)
                if j < 2:
                    for k in (4, 5):
                        emit_pk_dma(k, nc.sync)
                        emit_sp(k)
            for j in range(3, NPAIR):
                emit_pk_dma(2 * j, nc.sync)
                emit_sp(2 * j)
                emit_pk_dma(2 * j + 1, nc.sync)
                emit_sp(2 * j + 1)
                t = cap.tile([P, 2, GROUP, P], bf16, tag="ca", name=f"ca{j}")
                nc.sync.dma_start(out=t[:], in_=xga[:, 2 * j : 2 * j + 2])
                capair.append(t)
            xe0_sb = []
            for e in range(EXTRA):
                t = xep.tile([P, HALF, P], bf16, tag="xe0", name=f"xe0_{e}")
                nc.sync.dma_start(out=t[:], in_=xe0[e])
                xe0_sb.append(t)
            # extras' second halves (whole block: pass B's k16 row reads
            # all extras at once)
            xe1_sb = const.tile([P, HALF, EXTRA, P], bf16, tag="xe1", name="xe1")
            nc.sync.dma_start(out=xe1_sb[:], in_=xe1[:])
            # Second half: [pk, pk, cb_j] pairs for tiles 0..NB1-1.
            cbt = []
            for j in range(NPAIR):
                emit_pk_dma(HALF + 2 * j, nc.sync)
                emit_sp(HALF + 2 * j)
                emit_pk_dma(HALF + 2 * j + 1, nc.sync)
                emit_sp(HALF + 2 * j + 1)
                t = cbp.tile([P, 2, NB1, P], bf16, tag="cb", name=f"cb{j}")
                nc.sync.dma_start(out=t[:], in_=xgb[:, 2 * j : 2 * j + 2])
                cbt.append(t)

            def glhs(i, k):
                """lhsT AP for group tile i (0..NGRP-1), k-block k
                (B2 tiles 2..7 second halves are passed explicitly)."""
                if k < HALF:
                    if i < GROUP:
                        if k < 2:
                            return ca[k][:, 0, i, :]
                        return capair[(k - 2) // 2][:, (k - 2) % 2, i, :]
                    return xe0_sb[i - GROUP][:, k, :]
                if i < NB1:
                    return cbt[(k - HALF) // 2][:, (k - HALF) % 2, i, :]
                assert i >= GROUP
                return xe1_sb[:, k - HALF, i - GROUP, :]

            # ---- DVE construction for the first half (pass-A weights)
            for k in range(HALF):
                emit_wt(k)

            # Pass A: tiles 0-7, k 0..15, k-interleaved.
            pss = [
                psum_pool.tile([P, o_sh], f32, tag="ps", name=f"ps_a{bt}")
                for bt in range(GROUP)
            ]
            for k in range(HALF):
                for i in range(GROUP):
                    nc.tensor.matmul(
                        pss[i][:],
                        lhsT=glhs(i, k),
                        rhs=WT_bf[:, k, :],
                        start=(k == 0),
                        stop=(k == HALF - 1),
                    )
            parts = {}
            for i in range(GROUP):
                pa = part.tile([P, o_sh], bf16, tag="pA", name=f"pA_{i}")
                nc.vector.tensor_add(out=pa[:], in0=pss[i][:], in1=bias_sb[:])
                parts[i] = pa
            # A2: extras' first halves at full speed (W^T 0..HALF
            # resident); second-half constructions interleave on DVE so
            # each partsA2 eviction stays unblocked.
            wt_next = HALF
            for e in range(GROUP, NGRP):
                if wt_next < KB:
                    emit_wt(wt_next)
                    emit_wt(wt_next + 1)
                    wt_next += 2
                ps = psum_pool.tile([P, o_sh], f32, tag="ps", name=f"ps_a{e}")
                for k in range(HALF):
                    nc.tensor.matmul(
                        ps[:],
                        lhsT=glhs(e, k),
                        rhs=WT_bf[:, k, :],
                        start=(k == 0),
                        stop=(k == HALF - 1),
                    )
                pa = part.tile([P, o_sh], bf16, tag="pA", name=f"pA_{e}")
                nc.vector.tensor_add(out=pa[:], in0=ps[:], in1=bias_sb[:])
                parts[e] = pa
            for k in range(wt_next, KB):
                emit_wt(k)

            def split_tail(ps, pa, bt):
                y_sb = yout.tile([P, o_sh], f32, tag="y_sb")
                nc.vector.tensor_add(out=y_sb[:], in0=ps[:], in1=pa[:])
                nc.scalar.dma_start(out=y[bt * P : (bt + 1) * P, :], in_=y_sb[:])

            # B: extras + tiles 0..NB1-1, k 16..31, k-interleaved — the
            # delivery-paced pass rides the pk16-31 + cb chunk stream.
            b_tiles = list(range(GROUP, NGRP)) + list(range(NB1))
            psb = {
                i: psum_pool.tile([P, o_sh], f32, tag="ps", name=f"ps_b{i}")
                for i in b_tiles
            }
            for k in range(HALF, KB):
                for i in b_tiles:
                    nc.tensor.matmul(
                        psb[i][:],
                        lhsT=glhs(i, k),
                        rhs=WT_bf[:, k, :],
                        start=(k == HALF),
                        stop=(k == KB - 1),
                    )
            for i in b_tiles:
                split_tail(psb[i], parts[i], i)
            # B2: tiles NB1..7 second halves at full speed, off halves
            # delivered through the xin ring (its first allocations).
            for i in range(NB1, GROUP):
                hb = xin.tile([P, HALF, P], bf16, tag="xh", name=f"hb{i}")
                nc.sync.dma_start(out=hb[:], in_=xgc[i - NB1])
                ps = psum_pool.tile([P, o_sh], f32, tag="ps", name=f"ps_b{i}")
                for k in range(HALF, KB):
                    nc.tensor.matmul(
                        ps[:],
                        lhsT=hb[:, k - HALF, :],
                        rhs=WT_bf[:, k, :],
                        start=(k == HALF),
                        stop=(k == KB - 1),
                    )
                split_tail(ps, parts[i], i)

            # ---- remaining tiles stream one PSUM bank each; x arrives
            # as two half-tile DMAs through the xin ring.
            for bt in range(NGRP, BT):
                ha = xin.tile([P, HALF, P], bf16, tag="xh")
                hb = xin.tile([P, HALF, P], bf16, tag="xh")
                nc.sync.dma_start(out=ha[:], in_=xs[bt - NGRP, :, :HALF])
                nc.sync.dma_start(out=hb[:], in_=xs[bt - NGRP, :, HALF:])
                ps = psum_pool.tile([P, o_sh], f32, tag="ps")
                for k in range(KB):
                    xh = ha if k < HALF else hb
                    nc.tensor.matmul(
                        ps[:],
                        lhsT=xh[:, k % HALF, :],
                        rhs=WT_bf[:, k, :],
                        start=(k == 0),
                        stop=(k == KB - 1),
                    )
                y_sb = yout.tile([P, o_sh], f32, tag="y_sb")
                nc.vector.tensor_add(out=y_sb[:], in0=ps[:], in1=bias_sb[:])
                nc.scalar.dma_start(out=y[bt * P : (bt + 1) * P, :], in_=y_sb[:])

    # Skip bacc's pre-placed InstLoadActFuncSet: on large graphs walrus's
    # parallel-pass fork can separate the hoisted load from its activations
    # ("No Act func set exist for this instruction"); walrus's own lower_act
    # placement handles forked subgraphs correctly.
    nc.insert_act_table_loads = lambda: None
    nc.compile()
    return nc


def _prep_x(x):
    """[batch, in_f] fp32 -> bf16 tiles with layouts matching the DRAM
    tensors (x_t[..., pi, ..., bi] = x[bt*128 + bi, po*128 + pi]):
      xga [P, HALF, GROUP, P]    tiles 0-7,  k 0..15, k-major
      xgb [P, HALF, NB1, P]      tiles 0-1,  k 16..31, k-major
      xgc [6, P, HALF, P]        tiles 2-7,  k 16..31, tile-major
      xe0 [EXTRA, P, HALF, P]    extras 8-13, k 0..15, tile-major
      xe1 [P, HALF, EXTRA, P]    extras 8-13, k 16..31, k-major
      xs  [BT-NGRP, P, KB, P]    stream tiles, tile-major
    """
    batch, in_f = x.shape
    KB = in_f // P
    HALF = KB // 2
    BT = batch // P
    xbf = x.astype(ml_dtypes.bfloat16)
    xbf = xbf.reshape(BT, P, KB, P)  # [bt, bi, po, pi]
    xt = xbf.transpose(0, 3, 2, 1)  # [bt, pi, po, bi]
    xga = np.ascontiguousarray(xt[:GROUP, :, :HALF].transpose(1, 2, 0, 3))
    xgb = np.ascontiguousarray(xt[:NB1, :, HALF:].transpose(1, 2, 0, 3))
    xgc = np.ascontiguousarray(xt[NB1:GROUP, :, HALF:])
    xe0 = np.ascontiguousarray(xt[GROUP:NGRP, :, :HALF])
    xe1 = np.ascontiguousarray(xt[GROUP:NGRP, :, HALF:].transpose(1, 2, 0, 3))
    xs = np.ascontiguousarray(xt[NGRP:])
    return xga, xgb, xgc, xe0, xe1, xs


def _tile_w(w, dtype):
    """[o_sh, in_f] -> tiled [KB, 128, o_sh] with w_t[k, pi, o] = w[o, k*128 + pi]."""
    o_sh, in_f = w.shape
    return np.ascontiguousarray(w.T.reshape(in_f // P, P, o_sh)).astype(dtype)


def _prep_wpk(wmu, wrho, weps):
    """Pack mu (bf16), eps (bf16), rho (fp16 bits viewed as bf16) into one
    bf16-typed [KB, 128, 3*o_sh] tensor — one DMA per K-block."""
    mu = _tile_w(wmu, ml_dtypes.bfloat16)
    eps = _tile_w(weps, ml_dtypes.bfloat16)
    rho = _tile_w(wrho, np.float16).view(ml_dtypes.bfloat16)
    return np.ascontiguousarray(np.concatenate([mu, eps, rho], axis=2))


def make_in_maps(x, weight_mu, weight_rho, bias_mu, bias_rho, weight_eps, bias_eps):
    o_sh = OUT_F // N_CORES
    xga, xgb, xgc, xe0, xe1, xs = _prep_x(np.asarray(x, dtype=np.float32))
    wmu = np.asarray(weight_mu, dtype=np.float32)
    wrho = np.asarray(weight_rho, dtype=np.float32)
    weps = np.asarray(weight_eps, dtype=np.float32)
    bpk = np.stack(
        [
            np.asarray(bias_mu, dtype=np.float32),
            np.asarray(bias_rho, dtype=np.float32),
            np.asarray(bias_eps, dtype=np.float32),
        ]
    )  # [3, OUT_F]

    in_maps = []
    for c in range(N_CORES):
        rs = slice(c * o_sh, (c + 1) * o_sh)
        in_maps.append(
            {
                "x_ga": xga,
                "x_gb": xgb,
                "x_gc": xgc,
                "x_e0": xe0,
                "x_e1": xe1,
                "x_s": xs,
                "wpk_t": _prep_wpk(wmu[rs], wrho[rs], weps[rs]),
                "bias_pk": np.ascontiguousarray(bpk[:, rs].reshape(1, -1)),
            }
        )
    return in_maps


def kernel(x, weight_mu, weight_rho, bias_mu, bias_rho, weight_eps, bias_eps):
    o_sh = OUT_F // N_CORES
    key = (x.shape, o_sh)
    if key not in _NC_CACHE:
        _NC_CACHE[key] = build_nc(x.shape[0], x.shape[1], o_sh)
    nc = _NC_CACHE[key]

    in_maps = make_in_maps(
        x, weight_mu, weight_rho, bias_mu, bias_rho, weight_eps, bias_eps
    )
    res = run_bass_kernel_spmd(nc, in_maps, core_ids=list(range(N_CORES)))
    return np.concatenate([res.results[c]["y"] for c in range(N_CORES)], axis=1)
